# revision 9
# baseline (speedup 1.0000x reference)
"""Trainium2 Bass kernel for nn_FeatureRefinement.

Reference computation (bs=16, vl=1024, ql=64, d=1024):
    corr = einsum('bqd,bvd->bqv', Q, V); scores = softmax(corr, axis=1)
    corr_matrix = einsum('bqv,qd->bvd', scores, cor_w)     # cor_w constant over q
    sentence    = WeightedPool(Q)                           # (bs, d)
    sim         = cosine(V, sentence) + log(video_mask)     # (bs, vl)
    features    = concat([V, sim*sim_w, sentence_bcast, corr_matrix], -1)
    out         = relu(features @ mixer_w + mixer_b)

Algebraic restructuring (exact up to fp rounding):
  - softmax over q sums to 1  =>  corr_matrix[b,v,:] == cor_v_w*cor_q_w  (constant)
  - sim_features @ W2  == sim[b,v] * (sim_w.T @ W2)        (rank-1)
  - pooled_query @ W3  == sentence[b] @ W3                 (rank-1 per batch)
  so   out = relu(V @ W1 + [sim; 1; 1]^T @ [w2v; bias_hi; bias_lo])
  The only heavy compute is V @ W1 (4x FLOP reduction) plus O(bs*vl*d)
  vector work for the cosine similarity.

Sharding: data-parallel over batch, 2 batches per core on 8 cores. No
collectives; host scatters inputs / gathers outputs.

Implementation notes:
  - V^T is produced by the DMA xbar transpose engine (HBM->SBUF), not the
    PE.  The xbar writes V^T with d-index = p*KC + k (partition-major), so
    W1/W3/sentence chunk layouts are permuted identically (host reshape).
  - Row norms ||V[v]||^2 come from a second, untransposed read of V,
    squared+accumulated on the otherwise-idle GPSIMD engine.
  - Query side runs in fp16; alpha = Q @ pool_w is one fused GPSIMD op
    against a partition-broadcast pool_w row.
  - A short stream of junk matmuls at t=0 warms the PE HAM clock gate
    (cold PE runs at 1.2 GHz for its first ~3.4us of activity).
  - Output is stored fp16 and cast to fp32 on host (well within 2e-2).
"""
import sys

sys.path.insert(0, "/opt/trn_rl_repo")

import numpy as np
import ml_dtypes
from contextlib import ExitStack

import concourse.bass as bass
import concourse.tile as tile
from concourse import bacc, mybir
from concourse.bass_utils import run_bass_kernel_spmd
from concourse.masks import make_identity


def _install_ntff_shim():
    """This container's antenv lacks axon_hooks; if tracing is requested
    (BASS_TRACE=1), run_bass_kernel_spmd would crash importing it. Provide
    the hook via trn_agent_boot's ctypes helper, and keep the trace
    post-processing local (no bucket uploads)."""
    import types
    try:
        import antenv  # noqa: F401
        import antenv.axon_hooks  # noqa: F401
        return  # already present
    except ImportError:
        pass
    try:
        import trn_agent_boot.trn_boot as _tb
        hook = _tb._ntff_profile_via_ctypes("/opt/axon/libaxon_pjrt.so")
        mod = types.ModuleType("antenv.axon_hooks")
        mod.get_axon_ntff_profile_hook = lambda: hook
        sys.modules["antenv.axon_hooks"] = mod
        from concourse import bass_utils as _bu
        _orig = _bu.upload_artifacts

        def _safe_upload(tmpdir):
            try:
                return _orig(tmpdir)
            except Exception:
                return f"file://{tmpdir}"

        _bu.upload_artifacts = _safe_upload
    except Exception:
        pass


_install_ntff_shim()

F32 = mybir.dt.float32
F16 = mybir.dt.float16
BF16 = mybir.dt.bfloat16
AF = mybir.ActivationFunctionType
AX = mybir.AxisListType
ALU = mybir.AluOpType

BS, VL, QL, D = 16, 1024, 64, 1024
NCORES = 8
BPC = BS // NCORES          # batches per core
KC = D // 128               # contraction chunks
SS = 512                    # v-rows per super-slab
NSS = VL // SS              # super-slabs per batch
NEG_INF = -1e30

VDT = F16                   # dtype of the heavy V @ W1 path


def _build_program():
    nc = bacc.Bacc("TRN2", target_bir_lowering=False, debug=False, num_devices=NCORES)

    v_d = nc.dram_tensor("v", [BPC, VL, D], VDT, kind="ExternalInput").ap()
    q_d = nc.dram_tensor("q", [BPC, QL, D], F16, kind="ExternalInput").ap()
    qb_d = nc.dram_tensor("qb", [BPC, QL], F32, kind="ExternalInput").ap()
    vb_d = nc.dram_tensor("vb", [BPC, VL], F32, kind="ExternalInput").ap()
    w1_d = nc.dram_tensor("w1", [128, KC, D], VDT, kind="ExternalInput").ap()
    w3_d = nc.dram_tensor("w3", [128, KC, D], VDT, kind="ExternalInput").ap()
    w2v_d = nc.dram_tensor("w2v", [1, D], BF16, kind="ExternalInput").ap()
    biasc_d = nc.dram_tensor("biasc", [1, D], F32, kind="ExternalInput").ap()
    pw_d = nc.dram_tensor("pw", [1, D], F32, kind="ExternalInput").ap()
    out_d = nc.dram_tensor("out", [BPC, VL, D], F16, kind="ExternalOutput").ap()

    with tile.TileContext(nc) as tc, ExitStack() as ctx:
        singles = ctx.enter_context(tc.tile_pool(name="singles", bufs=1))
        qstuff = ctx.enter_context(tc.tile_pool(name="qstuff", bufs=1))
        rows = ctx.enter_context(tc.tile_pool(name="rows", bufs=2))
        vload = ctx.enter_context(tc.tile_pool(name="vload", bufs=4))
        trashp = ctx.enter_context(tc.tile_pool(name="trashp", bufs=2))
        opool = ctx.enter_context(tc.tile_pool(name="opool", bufs=3))
        psOut = ctx.enter_context(tc.tile_pool(name="psOut", bufs=4, space="PSUM"))
        psRow = ctx.enter_context(tc.tile_pool(name="psRow", bufs=3, space="PSUM"))
        psWarm = ctx.enter_context(tc.tile_pool(name="psWarm", bufs=1, space="PSUM"))

        # ================= t=0 DMA issues ==========================
        # HWDGE/sync: the four V^T xbar transposes (1 MiB each)
        vt = [[qstuff.tile([128, KC, SS], VDT, name=f"vt_{b}_{s}")
               for s in range(NSS)] for b in range(BPC)]
        for b in range(BPC):
            for s in range(NSS):
                nc.sync.dma_start_transpose(
                    out=vt[b][s], in_=v_d[b, s * SS:(s + 1) * SS, :])

        # HWDGE/scalar: the small query-side tensors
        q_tiles = []
        for b in range(BPC):
            q_sb = qstuff.tile([QL, D], F16, name=f"q{b}")
            nc.scalar.dma_start(out=q_sb, in_=q_d[b])
            q_tiles.append(q_sb)
        pw_sb = singles.tile([1, D], F32)
        nc.scalar.dma_start(out=pw_sb, in_=pw_d)
        biasc_sb = singles.tile([1, D], F32)
        nc.scalar.dma_start(out=biasc_sb, in_=biasc_d)
        qb_sb = qstuff.tile([1, BPC, QL], F32)
        vb_sb = qstuff.tile([1, BPC, VL], F32)
        for b in range(BPC):
            nc.scalar.dma_start(out=qb_sb[:, b, :], in_=qb_d[b:b + 1, :])
            nc.scalar.dma_start(out=vb_sb[:, b, :], in_=vb_d[b:b + 1, :])
        aug3 = [qstuff.tile([3, D], BF16, name=f"aug3_{b}") for b in range(BPC)]
        for b in range(BPC):
            nc.scalar.dma_start(out=aug3[b][0:1, :], in_=w2v_d)

        # SWDGE/gpsimd: V rows (for norms), W1, W3 — ordered by deadline
        v_sb_all = {}
        for s4 in range(4):  # batch 0, chunks 0-3 (needed ~7us)
            v_sb = vload.tile([128, D], VDT, tag="v_sb", name=f"v_0_{s4}")
            nc.gpsimd.dma_start(out=v_sb, in_=v_d[0, s4 * 128:(s4 + 1) * 128, :])
            v_sb_all[(0, s4)] = v_sb
        w1_sb = singles.tile([128, KC, D], VDT)
        w3ctx = ExitStack()
        w3pool = w3ctx.enter_context(tc.tile_pool(name="w3pool", bufs=1))
        w3_sb = w3pool.tile([128, KC, D], VDT)
        nc.gpsimd.dma_start(out=w1_sb[:, :, 0:512], in_=w1_d[:, :, 0:512])
        nc.gpsimd.dma_start(out=w3_sb[:, :, 0:512], in_=w3_d[:, :, 0:512])
        nc.gpsimd.dma_start(out=w1_sb[:, :, 512:D], in_=w1_d[:, :, 512:D])
        nc.gpsimd.dma_start(out=w3_sb[:, :, 512:D], in_=w3_d[:, :, 512:D])
        for b in range(BPC):
            for s4 in range(8):
                if (b, s4) in v_sb_all:
                    continue
                v_sb = vload.tile([128, D], VDT, tag="v_sb", name=f"v_{b}_{s4}")
                nc.gpsimd.dma_start(out=v_sb, in_=v_d[b, s4 * 128:(s4 + 1) * 128, :])
                v_sb_all[(b, s4)] = v_sb

        # ================= HAM warmup ==============================
        warm16 = singles.tile([128, 512], F16)
        nc.vector.memset(warm16, 0.0)
        warm_ps = psWarm.tile([128, 512], F32)
        for _ in range(12):
            nc.tensor.matmul(warm_ps, warm16[:, 0:128], warm16,
                             start=True, stop=True)

        # identity (only small fp32 transposes use it)
        ident = singles.tile([128, 128], F32)
        make_identity(nc, ident)

        # pool_w broadcast to QL partitions for the fused alpha op
        pw64 = singles.tile([QL, D], F32)
        nc.gpsimd.partition_broadcast(pw64, pw_sb)

        # ================= Phase A: query side =====================
        sentT2 = qstuff.tile([128, KC, BPC], VDT)    # sentence^T, permuted d
        snsq2 = qstuff.tile([1, BPC], F32)           # clamped ||sentence||^2

        for b in range(BPC):
            q_sb = q_tiles[b]
            # alpha[q] = sum_d Q[q,d]*pw[d]  (one fused gpsimd op)
            qtrash = trashp.tile([QL, D], F16, tag="qtrash")
            alpha_col = rows.tile([QL, 1], F32)
            nc.vector.scalar_tensor_tensor(
                out=qtrash, in0=q_sb, scalar=1.0, in1=pw64,
                op0=ALU.mult, op1=ALU.mult, accum_out=alpha_col)
            al_ps = psRow.tile([1, QL], F32, tag="row")
            nc.tensor.transpose(al_ps, alpha_col, ident[:QL, :QL])
            alpha_sb = rows.tile([1, QL], F32)
            nc.vector.tensor_add(alpha_sb, al_ps, qb_sb[:, b, :])

            # softmax over the free dim (1 partition)
            mx = rows.tile([1, 1], F32)
            nc.vector.reduce_max(mx, alpha_sb, axis=AX.X)
            asub = rows.tile([1, QL], F32)
            nc.vector.tensor_scalar_sub(asub, alpha_sb, mx)
            aexp = rows.tile([1, QL], F32)
            asum = rows.tile([1, 1], F32)
            nc.scalar.activation(aexp, asub, AF.Exp, accum_out=asum)
            rsum = rows.tile([1, 1], F32)
            nc.vector.reciprocal(rsum, asum)
            alphas_sb = rows.tile([1, QL], F32)
            nc.vector.tensor_scalar_mul(alphas_sb, aexp, rsum)

            # alphas^T : [QL, 1] fp16 (lhsT of the sentence matmul)
            alT_ps = psRow.tile([QL, 1], F32, tag="row")
            nc.tensor.transpose(alT_ps, alphas_sb, ident[:1, :1])
            alphasT_sb = rows.tile([QL, 1], F16)
            nc.vector.tensor_copy(alphasT_sb, alT_ps)

            # sentence = alphas @ Q : [1, D] fp32
            sent_sb = rows.tile([1, D], F32, tag="sent", bufs=1)
            for h in range(2):
                s_ps = psRow.tile([1, 512], F32, tag="row")
                nc.tensor.matmul(s_ps, alphasT_sb, q_sb[:, h * 512:(h + 1) * 512],
                                 start=True, stop=True)
                nc.vector.tensor_copy(sent_sb[:, h * 512:(h + 1) * 512], s_ps)

            # ||sentence||^2 clamped
            strash = rows.tile([1, D], F32, tag="strash", bufs=1)
            ssq = rows.tile([1, 1], F32)
            nc.scalar.activation(strash, sent_sb, AF.Square, accum_out=ssq)
            nc.vector.tensor_scalar_max(snsq2[:, b:b + 1], ssq, 1e-16)

            # sentence^T chunks: sentT2[p,k] = sent[k*128+p] (xbar layout)
            sT_ps = psRow.tile([128, KC], F32, tag="row")
            for k in range(KC):
                nc.tensor.transpose(sT_ps[:, k:k + 1],
                                    sent_sb[:, k * 128:(k + 1) * 128],
                                    ident[:1, :1])
            nc.vector.tensor_copy(sentT2[:, :, b], sT_ps)

        # bias rows, both batches at once (M=2):
        #   bias_f[b] = sentence[b] @ W3 + biasc, split bf16 hi+lo
        biasc2 = singles.tile([BPC, D], F32)
        nc.gpsimd.partition_broadcast(biasc2, biasc_sb)
        bias_f = rows.tile([2, D], F32, tag="biasf", bufs=1)
        for h in range(2):
            b_ps = psRow.tile([2, 512], F32, tag="row")
            for k in range(KC):
                nc.tensor.matmul(b_ps, sentT2[:, k, 0:BPC],
                                 w3_sb[:, k, h * 512:(h + 1) * 512],
                                 start=(k == 0), stop=(k == KC - 1))
            nc.vector.tensor_add(bias_f[:, h * 512:(h + 1) * 512], b_ps,
                                 biasc2[:, h * 512:(h + 1) * 512])
        bias_hi = rows.tile([2, D], BF16, tag="biashi", bufs=1)
        nc.vector.tensor_copy(bias_hi, bias_f)
        bias_lo = rows.tile([2, D], BF16, tag="biaslo", bufs=1)
        nc.vector.tensor_sub(bias_lo, bias_f, bias_hi)
        # engines can't write partitions 1:3 of aug3 directly; bounce the
        # bias rows through a DRAM scratch tile (DMA has no such limit)
        dramp = ctx.enter_context(tc.tile_pool(name="dramp", bufs=1, space="DRAM"))
        augd = dramp.tile([BPC, 2, D], BF16)
        nc.gpsimd.dma_start(out=augd[:, 0, :], in_=bias_hi)
        nc.gpsimd.dma_start(out=augd[:, 1, :], in_=bias_lo)
        for b in range(BPC):
            nc.gpsimd.dma_start(out=aug3[b][1:3, :], in_=augd[b])
        w3ctx.close()  # release W3's SBUF before the heavy phase

        # augment lhsT tiles: rows 1:3 are the constant ones
        aug_l = [[qstuff.tile([3, SS], BF16, name=f"augl_{b}_{s}")
                  for s in range(NSS)] for b in range(BPC)]
        for b in range(BPC):
            for s in range(NSS):
                nc.vector.memset(aug_l[b][s], 1.0)  # row 0 overwritten by sim

        # row norms ||V[v]||^2 on gpsimd (square + free-axis accumulate)
        vnsq = {}
        for b in range(BPC):
            for s in range(NSS):
                vnsq[(b, s)] = qstuff.tile([128, 4], F32, name=f"vnsq_{b}_{s}")
        for b in range(BPC):
            for s4 in range(8):
                vtr = trashp.tile([128, D], F16, tag="vtrash")
                nc.vector.scalar_tensor_tensor(
                    out=vtr, in0=v_sb_all[(b, s4)], scalar=1.0,
                    in1=v_sb_all[(b, s4)], op0=ALU.mult, op1=ALU.mult,
                    accum_out=vnsq[(b, s4 // 4)][:, s4 % 4:s4 % 4 + 1])

        # ================= Phase C: video side (heavy) =============
        for b in range(BPC):
            for s in range(NSS):
                vts = vt[b][s]
                # dot row: sentence . V^T  -> [1, SS]
                dot_ps = psRow.tile([1, SS], F32, tag="row")
                for k in range(KC):
                    nc.tensor.matmul(dot_ps, sentT2[:, k, b:b + 1], vts[:, k, :],
                                     start=(k == 0), stop=(k == KC - 1))
                # norm row
                vnr_ps = psRow.tile([1, SS], F32, tag="row")
                for s4 in range(4):
                    nc.tensor.transpose(vnr_ps[:, s4 * 128:(s4 + 1) * 128],
                                        vnsq[(b, s)][:, s4:s4 + 1], ident)

                # sim = dot * rsqrt(max(vnsq,eps)*snsq) + log(video_mask)
                t1 = rows.tile([1, SS], F32, tag="t1")
                nc.vector.tensor_scalar(t1, vnr_ps, 1e-16, snsq2[:, b:b + 1],
                                        op0=ALU.max, op1=ALU.mult)
                t3 = rows.tile([1, SS], F32, tag="t3")
                nc.scalar.activation(t3, t1, AF.Abs_reciprocal_sqrt)
                t4 = rows.tile([1, SS], F32, tag="t4")
                nc.vector.tensor_mul(t4, dot_ps, t3)
                nc.vector.tensor_add(aug_l[b][s][0:1, :], t4,
                                     vb_sb[:, b, s * SS:(s + 1) * SS])

                for i in range(4):
                    out_sb = opool.tile([128, D], F16)
                    o_ps = [psOut.tile([128, 512], F32, tag="o_ps",
                                       name=f"o_ps_{b}_{s}_{i}_{h}")
                            for h in range(2)]
                    # keep 8 consecutive MMs on one PSUM bank: per-instruction
                    # bank alternation triggers the PE depth-cycling penalty
                    for h in range(2):
                        for k in range(KC):
                            nc.tensor.matmul(
                                o_ps[h], vts[:, k, i * 128:(i + 1) * 128],
                                w1_sb[:, k, h * 512:(h + 1) * 512],
                                start=(k == 0), stop=False)
                    for h in range(2):
                        nc.tensor.matmul(
                            o_ps[h], aug_l[b][s][:, i * 128:(i + 1) * 128],
                            aug3[b][:, h * 512:(h + 1) * 512],
                            start=False, stop=True)
                        # relu on DVE (fp16 store)
                        nc.vector.tensor_scalar_max(
                            out_sb[:, h * 512:(h + 1) * 512], o_ps[h], 0.0)
                    r0 = s * SS + i * 128
                    nc.sync.dma_start(out=out_d[b, r0:r0 + 128, :], in_=out_sb)

    nc.compile()
    return nc


_NC = None
_LAST_RESULTS = None


def _get_program():
    global _NC
    if _NC is None:
        _NC = _build_program()
    return _NC


def kernel(video_features, query_features, video_mask, query_mask,
           sim_w, cor_v_w, cor_q_w, pool_w, mixer_w, mixer_b):
    video_features = np.asarray(video_features, dtype=np.float32)
    query_features = np.asarray(query_features, dtype=np.float32)
    video_mask = np.asarray(video_mask, dtype=np.float32)
    query_mask = np.asarray(query_mask, dtype=np.float32)
    sim_w = np.asarray(sim_w, dtype=np.float32)
    cor_v_w = np.asarray(cor_v_w, dtype=np.float32)
    cor_q_w = np.asarray(cor_q_w, dtype=np.float32)
    pool_w = np.asarray(pool_w, dtype=np.float32)
    mixer_w = np.asarray(mixer_w, dtype=np.float32)
    mixer_b = np.asarray(mixer_b, dtype=np.float32)

    # host-side folds of the weight-only algebra (O(d^2), negligible).
    # W1/W3 reshaped to the per-chunk layout w1[p, k, n] = W1[k*128+p, n]
    # matching the xbar-transposed V^T chunks.
    W1p = np.ascontiguousarray(
        mixer_w[0:D].reshape(KC, 128, D).transpose(1, 0, 2)).astype(np.float16)
    W2 = mixer_w[D:2 * D]
    W3p = np.ascontiguousarray(
        mixer_w[2 * D:3 * D].reshape(KC, 128, D).transpose(1, 0, 2)).astype(np.float16)
    W4 = mixer_w[3 * D:4 * D]
    w2v = (sim_w[:, 0] @ W2.astype(np.float32)).astype(ml_dtypes.bfloat16)[None, :]
    cor_vec = (cor_v_w[0] * cor_q_w[0, 0]).astype(np.float32)
    biasc = (cor_vec @ W4 + mixer_b).astype(np.float32)[None, :]
    qbias = ((1.0 - query_mask) * NEG_INF).astype(np.float32)
    vbias = np.log(video_mask + 1e-45).astype(np.float32)
    pw_row = np.ascontiguousarray(pool_w[:, 0])[None, :]  # [1, D]
    v16 = video_features.astype(np.float16)
    q16 = query_features.astype(np.float16)

    nc = _get_program()
    in_maps = []
    for c in range(NCORES):
        sl = slice(c * BPC, (c + 1) * BPC)
        in_maps.append({
            "v": np.ascontiguousarray(v16[sl]),
            "q": np.ascontiguousarray(q16[sl]),
            "qb": np.ascontiguousarray(qbias[sl]),
            "vb": np.ascontiguousarray(vbias[sl]),
            "w1": W1p,
            "w3": W3p,
            "w2v": w2v,
            "biasc": biasc,
            "pw": pw_row,
        })
    res = run_bass_kernel_spmd(nc, in_maps, core_ids=list(range(NCORES)))
    global _LAST_RESULTS
    _LAST_RESULTS = res
    out = np.concatenate([res.results[c]["out"] for c in range(NCORES)], axis=0)
    return out.astype(np.float32)


# revision 11
# speedup vs baseline: 1.3015x; 1.3015x over previous
"""Trainium2 Bass kernel for nn_FeatureRefinement.

Reference computation (bs=16, vl=1024, ql=64, d=1024):
    corr = einsum('bqd,bvd->bqv', Q, V); scores = softmax(corr, axis=1)
    corr_matrix = einsum('bqv,qd->bvd', scores, cor_w)     # cor_w constant over q
    sentence    = WeightedPool(Q)                           # (bs, d)
    sim         = cosine(V, sentence) + log(video_mask)     # (bs, vl)
    features    = concat([V, sim*sim_w, sentence_bcast, corr_matrix], -1)
    out         = relu(features @ mixer_w + mixer_b)

Algebraic restructuring (exact up to fp rounding):
  - softmax over q sums to 1  =>  corr_matrix[b,v,:] == cor_v_w*cor_q_w  (constant)
  - sim_features @ W2  == sim[b,v] * (sim_w.T @ W2)        (rank-1)
  - pooled_query @ W3  == sentence[b] @ W3                 (rank-1 per batch)
  so   out = relu(V @ W1 + [sim; 1; 1]^T @ [w2v; bias_hi; bias_lo])
  The only heavy compute is V @ W1 (4x FLOP reduction) plus O(bs*vl*d)
  vector work for the cosine similarity.

Sharding: data-parallel over batch, 2 batches per core on 8 cores. No
collectives; host scatters inputs / gathers outputs.

Implementation notes:
  - Query side runs in fp16; alpha = Q @ pool_w is one fused DVE op
    against a partition-broadcast pool_w row (no Q^T transposes).
  - Bias rows for both batches are computed in one M=2 matmul group and
    bounced through a DRAM scratch tile into the 3-partition augment rhs
    (engines cannot address partition offsets 1-2 directly).
  - A short stream of junk matmuls at t=0 warms the PE HAM clock gate
    (a cold PE runs at 1.2 GHz for its first ~3.4us of activity).
  - Output is stored fp16 and cast to fp32 on host (well within 2e-2).
  - DMA queue budget (per-queue, not per-link, is the constraint):
    sync carries V (4 MiB @ ~133 GB/s), gpsimd carries W1 (2 MiB @ ~173),
    scalar carries W3 + small tensors early and the fp16 stores late.
"""
import sys

sys.path.insert(0, "/opt/trn_rl_repo")

import numpy as np
import ml_dtypes
from contextlib import ExitStack

import concourse.bass as bass
import concourse.tile as tile
from concourse import bacc, mybir
from concourse.bass_utils import run_bass_kernel_spmd
from concourse.masks import make_identity


def _install_ntff_shim():
    """This container's antenv lacks axon_hooks; if tracing is requested
    (BASS_TRACE=1), run_bass_kernel_spmd would crash importing it. Provide
    the hook via trn_agent_boot's ctypes helper, and keep the trace
    post-processing local (no bucket uploads)."""
    import types
    try:
        import antenv  # noqa: F401
        import antenv.axon_hooks  # noqa: F401
        return  # already present
    except ImportError:
        pass
    try:
        import trn_agent_boot.trn_boot as _tb
        hook = _tb._ntff_profile_via_ctypes("/opt/axon/libaxon_pjrt.so")
        mod = types.ModuleType("antenv.axon_hooks")
        mod.get_axon_ntff_profile_hook = lambda: hook
        sys.modules["antenv.axon_hooks"] = mod
        from concourse import bass_utils as _bu
        _orig = _bu.upload_artifacts

        def _safe_upload(tmpdir):
            try:
                return _orig(tmpdir)
            except Exception:
                return f"file://{tmpdir}"

        _bu.upload_artifacts = _safe_upload
    except Exception:
        pass


_install_ntff_shim()

F32 = mybir.dt.float32
F16 = mybir.dt.float16
BF16 = mybir.dt.bfloat16
AF = mybir.ActivationFunctionType
AX = mybir.AxisListType
ALU = mybir.AluOpType

BS, VL, QL, D = 16, 1024, 64, 1024
NCORES = 8
BPC = BS // NCORES          # batches per core
KC = D // 128               # contraction chunks
SS = 512                    # v-rows per super-slab
NSS = VL // SS              # super-slabs per batch
NEG_INF = -1e30

VDT = F16                   # dtype of the heavy V @ W1 path


def _build_program():
    nc = bacc.Bacc("TRN2", target_bir_lowering=False, debug=False, num_devices=NCORES)

    v_d = nc.dram_tensor("v", [BPC, VL, D], VDT, kind="ExternalInput").ap()
    q_d = nc.dram_tensor("q", [BPC, QL, D], F16, kind="ExternalInput").ap()
    qb_d = nc.dram_tensor("qb", [BPC, QL], F32, kind="ExternalInput").ap()
    vb_d = nc.dram_tensor("vb", [BPC, VL], F32, kind="ExternalInput").ap()
    w1_d = nc.dram_tensor("w1", [128, KC, D], VDT, kind="ExternalInput").ap()
    w3_d = nc.dram_tensor("w3", [128, KC, D], VDT, kind="ExternalInput").ap()
    w2v_d = nc.dram_tensor("w2v", [1, D], BF16, kind="ExternalInput").ap()
    biasc_d = nc.dram_tensor("biasc", [1, D], F32, kind="ExternalInput").ap()
    pw_d = nc.dram_tensor("pw", [1, D], F32, kind="ExternalInput").ap()
    out_d = nc.dram_tensor("out", [BPC, VL, D], F16, kind="ExternalOutput").ap()

    with tile.TileContext(nc) as tc, ExitStack() as ctx:
        singles = ctx.enter_context(tc.tile_pool(name="singles", bufs=1))
        qstuff = ctx.enter_context(tc.tile_pool(name="qstuff", bufs=1))
        rows = ctx.enter_context(tc.tile_pool(name="rows", bufs=2))
        vload = ctx.enter_context(tc.tile_pool(name="vload", bufs=8))
        trashp = ctx.enter_context(tc.tile_pool(name="trashp", bufs=2))
        psA = ctx.enter_context(tc.tile_pool(name="psA", bufs=2, space="PSUM"))
        psOut = ctx.enter_context(tc.tile_pool(name="psOut", bufs=4, space="PSUM"))
        psRow = ctx.enter_context(tc.tile_pool(name="psRow", bufs=2, space="PSUM"))
        dramp = ctx.enter_context(tc.tile_pool(name="dramp", bufs=1, space="DRAM"))

        # ================= t=0 DMA issues ==========================
        # sync queue: all V row loads (consumed c1-order; vload recycles)
        v_sb_all = {}
        for b in range(BPC):
            for s4 in range(8):
                v_sb = vload.tile([128, D], VDT, tag="v_sb", name=f"v_{b}_{s4}")
                nc.sync.dma_start(out=v_sb, in_=v_d[b, s4 * 128:(s4 + 1) * 128, :])
                v_sb_all[(b, s4)] = v_sb

        # scalar queue: query-side smalls, then W3
        q_tiles = []
        for b in range(BPC):
            q_sb = qstuff.tile([QL, D], F16, name=f"q{b}")
            nc.scalar.dma_start(out=q_sb, in_=q_d[b])
            q_tiles.append(q_sb)
        pw_sb = singles.tile([1, D], F32)
        nc.scalar.dma_start(out=pw_sb, in_=pw_d)
        biasc_sb = singles.tile([1, D], F32)
        nc.scalar.dma_start(out=biasc_sb, in_=biasc_d)
        qb_sb = qstuff.tile([1, BPC, QL], F32)
        vb_sb = qstuff.tile([1, BPC, VL], F32)
        for b in range(BPC):
            nc.scalar.dma_start(out=qb_sb[:, b, :], in_=qb_d[b:b + 1, :])
            nc.scalar.dma_start(out=vb_sb[:, b, :], in_=vb_d[b:b + 1, :])
        aug3 = [qstuff.tile([3, D], BF16, name=f"aug3_{b}") for b in range(BPC)]
        for b in range(BPC):
            nc.scalar.dma_start(out=aug3[b][0:1, :], in_=w2v_d)
        w3ctx = ExitStack()
        w3pool = w3ctx.enter_context(tc.tile_pool(name="w3pool", bufs=1))
        w3_sb = w3pool.tile([128, KC, D], VDT)
        nc.scalar.dma_start(out=w3_sb[:, :, 0:512], in_=w3_d[:, :, 0:512])
        nc.scalar.dma_start(out=w3_sb[:, :, 512:D], in_=w3_d[:, :, 512:D])

        # gpsimd queue: W1
        w1_sb = singles.tile([128, KC, D], VDT)
        nc.gpsimd.dma_start(out=w1_sb[:, :, 0:512], in_=w1_d[:, :, 0:512])
        nc.gpsimd.dma_start(out=w1_sb[:, :, 512:D], in_=w1_d[:, :, 512:D])

        # ================= HAM warmup ==============================
        warm16 = singles.tile([128, 512], F16)
        nc.vector.memset(warm16, 0.0)
        for r in range(12):
            warm_ps = psOut.tile([128, 512], F32, tag="o_ps", name=f"warm{r}")
            nc.tensor.matmul(warm_ps, warm16[:, 0:128], warm16,
                             start=True, stop=True)

        # identities
        ident = singles.tile([128, 128], F32)
        make_identity(nc, ident)
        identH = singles.tile([128, 128], VDT)
        nc.vector.tensor_copy(identH, ident)

        # pool_w broadcast to QL partitions for the fused alpha op
        pw64 = singles.tile([QL, D], F32)
        nc.gpsimd.partition_broadcast(pw64, pw_sb)

        # ================= Phase A: query side =====================
        sentT2 = qstuff.tile([128, KC, BPC], VDT)    # sentence^T chunks
        snsq2 = qstuff.tile([1, BPC], F32)           # clamped ||sentence||^2

        for b in range(BPC):
            q_sb = q_tiles[b]
            # alpha[q] = sum_d Q[q,d]*pw[d]  (one fused DVE op)
            qtrash = trashp.tile([QL, D], F16, tag="qtrash")
            alpha_col = rows.tile([QL, 1], F32)
            nc.vector.scalar_tensor_tensor(
                out=qtrash, in0=q_sb, scalar=1.0, in1=pw64,
                op0=ALU.mult, op1=ALU.mult, accum_out=alpha_col)
            al_ps = psRow.tile([1, QL], F32, tag="row")
            nc.tensor.transpose(al_ps, alpha_col, ident[:QL, :QL])
            alpha_sb = rows.tile([1, QL], F32)
            nc.vector.tensor_add(alpha_sb, al_ps, qb_sb[:, b, :])

            # softmax over the free dim (1 partition)
            mx = rows.tile([1, 1], F32)
            nc.vector.reduce_max(mx, alpha_sb, axis=AX.X)
            asub = rows.tile([1, QL], F32)
            nc.vector.tensor_scalar_sub(asub, alpha_sb, mx)
            aexp = rows.tile([1, QL], F32)
            asum = rows.tile([1, 1], F32)
            nc.scalar.activation(aexp, asub, AF.Exp, accum_out=asum)
            rsum = rows.tile([1, 1], F32)
            nc.vector.reciprocal(rsum, asum)
            alphas_sb = rows.tile([1, QL], F32)
            nc.vector.tensor_scalar_mul(alphas_sb, aexp, rsum)

            # alphas^T : [QL, 1] fp16 (lhsT of the sentence matmul)
            alT_ps = psRow.tile([QL, 1], F32, tag="row")
            nc.tensor.transpose(alT_ps, alphas_sb, ident[:1, :1])
            alphasT_sb = rows.tile([QL, 1], F16)
            nc.vector.tensor_copy(alphasT_sb, alT_ps)

            # sentence = alphas @ Q : [1, D] fp32
            sent_sb = rows.tile([1, D], F32, tag="sent", bufs=1)
            for h in range(2):
                s_ps = psRow.tile([1, 512], F32, tag="row")
                nc.tensor.matmul(s_ps, alphasT_sb, q_sb[:, h * 512:(h + 1) * 512],
                                 start=True, stop=True)
                nc.vector.tensor_copy(sent_sb[:, h * 512:(h + 1) * 512], s_ps)

            # ||sentence||^2 clamped
            strash = rows.tile([1, D], F32, tag="strash", bufs=1)
            ssq = rows.tile([1, 1], F32)
            nc.scalar.activation(strash, sent_sb, AF.Square, accum_out=ssq)
            nc.vector.tensor_scalar_max(snsq2[:, b:b + 1], ssq, 1e-16)

            # sentence^T chunks: sentT2[p,k] = sent[k*128+p]
            sT_ps = psRow.tile([128, KC], F32, tag="row")
            for k in range(KC):
                nc.tensor.transpose(sT_ps[:, k:k + 1],
                                    sent_sb[:, k * 128:(k + 1) * 128],
                                    ident[:1, :1])
            nc.vector.tensor_copy(sentT2[:, :, b], sT_ps)

        # bias rows, both batches at once (M=2):
        #   bias_f[b] = sentence[b] @ W3 + biasc, split bf16 hi+lo
        biasc2 = singles.tile([BPC, D], F32)
        nc.gpsimd.partition_broadcast(biasc2, biasc_sb)
        bias_f = rows.tile([2, D], F32, tag="biasf", bufs=1)
        for h in range(2):
            b_ps = psRow.tile([2, 512], F32, tag="row")
            for k in range(KC):
                nc.tensor.matmul(b_ps, sentT2[:, k, 0:BPC],
                                 w3_sb[:, k, h * 512:(h + 1) * 512],
                                 start=(k == 0), stop=(k == KC - 1))
            nc.vector.tensor_add(bias_f[:, h * 512:(h + 1) * 512], b_ps,
                                 biasc2[:, h * 512:(h + 1) * 512])
        bias_hi = rows.tile([2, D], BF16, tag="biashi", bufs=1)
        nc.vector.tensor_copy(bias_hi, bias_f)
        bias_lo = rows.tile([2, D], BF16, tag="biaslo", bufs=1)
        nc.vector.tensor_sub(bias_lo, bias_f, bias_hi)
        # engines can't write partitions 1:3 of aug3 directly; bounce the
        # bias rows through a DRAM scratch tile (DMA has no such limit)
        augd = dramp.tile([BPC, 2, D], BF16)
        nc.gpsimd.dma_start(out=augd[:, 0, :], in_=bias_hi)
        nc.gpsimd.dma_start(out=augd[:, 1, :], in_=bias_lo)
        for b in range(BPC):
            nc.gpsimd.dma_start(out=aug3[b][1:3, :], in_=augd[b])
        w3ctx.close()  # release W3's SBUF before the heavy phase

        # augment lhsT tiles: rows 1:3 are the constant ones
        aug_l = [[qstuff.tile([3, SS], BF16, name=f"augl_{b}_{s}")
                  for s in range(NSS)] for b in range(BPC)]
        for b in range(BPC):
            for s in range(NSS):
                nc.vector.memset(aug_l[b][s], 1.0)  # row 0 overwritten by sim

        # ================= Phase C: video side (heavy) =============
        vtpool = ctx.enter_context(tc.tile_pool(name="vtpool", bufs=4))
        opool = ctx.enter_context(tc.tile_pool(name="opool", bufs=3))

        for b in range(BPC):
            # --- C1: row norms + transpose into vt (both super-slabs)
            vts, vnsqs = [], []
            for s in range(NSS):
                vt = vtpool.tile([128, KC, SS], VDT, tag="vt", name=f"vt_{b}_{s}")
                vnsq_col = rows.tile([128, 4], F32, tag="vnsqc")
                for s4 in range(4):
                    v_sb = v_sb_all[(b, s * 4 + s4)]
                    vtrash = trashp.tile([128, D], F32, tag="vtrash")
                    nc.scalar.activation(vtrash, v_sb, AF.Square,
                                         accum_out=vnsq_col[:, s4:s4 + 1])
                    for g in range(2):
                        t_ps = psA.tile([128, 512], VDT, tag="tps")
                        for j in range(4):
                            k = g * 4 + j
                            nc.tensor.transpose(
                                t_ps[:, j * 128:(j + 1) * 128],
                                v_sb[:, k * 128:(k + 1) * 128], identH)
                        nc.vector.tensor_copy(
                            vt[:, g * 4:(g + 1) * 4, s4 * 128:(s4 + 1) * 128],
                            t_ps.rearrange("p (j c) -> p j c", j=4))
                vts.append(vt)
                vnsqs.append(vnsq_col)

            # --- C2: sim row + main matmuls per super-slab
            for s in range(NSS):
                vt, vnsq_col = vts[s], vnsqs[s]
                # dot row: sentence . V^T  -> [1, SS]
                dot_ps = psRow.tile([1, SS], F32, tag="row")
                for k in range(KC):
                    nc.tensor.matmul(dot_ps, sentT2[:, k, b:b + 1], vt[:, k, :],
                                     start=(k == 0), stop=(k == KC - 1))
                vnr_ps = psRow.tile([1, SS], F32, tag="row")
                for s4 in range(4):
                    nc.tensor.transpose(vnr_ps[:, s4 * 128:(s4 + 1) * 128],
                                        vnsq_col[:, s4:s4 + 1], ident)

                # sim = dot * rsqrt(max(vnsq,eps)*snsq) + log(video_mask)
                t1 = rows.tile([1, SS], F32, tag="t1")
                nc.vector.tensor_scalar(t1, vnr_ps, 1e-16, snsq2[:, b:b + 1],
                                        op0=ALU.max, op1=ALU.mult)
                t3 = rows.tile([1, SS], F32, tag="t3")
                nc.scalar.activation(t3, t1, AF.Abs_reciprocal_sqrt)
                t4 = rows.tile([1, SS], F32, tag="t4")
                nc.vector.tensor_mul(t4, dot_ps, t3)
                nc.vector.tensor_add(aug_l[b][s][0:1, :], t4,
                                     vb_sb[:, b, s * SS:(s + 1) * SS])

                for i in range(4):
                    out_sb = opool.tile([128, D], F16)
                    o_ps = [psOut.tile([128, 512], F32, tag="o_ps",
                                       name=f"o_ps_{b}_{s}_{i}_{h}")
                            for h in range(2)]
                    # keep 8 consecutive MMs on one PSUM bank: per-instruction
                    # bank alternation triggers the PE depth-cycling penalty
                    for h in range(2):
                        for k in range(KC):
                            nc.tensor.matmul(
                                o_ps[h], vt[:, k, i * 128:(i + 1) * 128],
                                w1_sb[:, k, h * 512:(h + 1) * 512],
                                start=(k == 0), stop=False)
                    for h in range(2):
                        nc.tensor.matmul(
                            o_ps[h], aug_l[b][s][:, i * 128:(i + 1) * 128],
                            aug3[b][:, h * 512:(h + 1) * 512],
                            start=False, stop=True)
                        # relu on DVE (fp16 store)
                        nc.vector.tensor_scalar_max(
                            out_sb[:, h * 512:(h + 1) * 512], o_ps[h], 0.0)
                    r0 = s * SS + i * 128
                    nc.scalar.dma_start(out=out_d[b, r0:r0 + 128, :], in_=out_sb)

    nc.compile()
    return nc


_NC = None
_LAST_RESULTS = None


def _get_program():
    global _NC
    if _NC is None:
        _NC = _build_program()
    return _NC


def kernel(video_features, query_features, video_mask, query_mask,
           sim_w, cor_v_w, cor_q_w, pool_w, mixer_w, mixer_b):
    video_features = np.asarray(video_features, dtype=np.float32)
    query_features = np.asarray(query_features, dtype=np.float32)
    video_mask = np.asarray(video_mask, dtype=np.float32)
    query_mask = np.asarray(query_mask, dtype=np.float32)
    sim_w = np.asarray(sim_w, dtype=np.float32)
    cor_v_w = np.asarray(cor_v_w, dtype=np.float32)
    cor_q_w = np.asarray(cor_q_w, dtype=np.float32)
    pool_w = np.asarray(pool_w, dtype=np.float32)
    mixer_w = np.asarray(mixer_w, dtype=np.float32)
    mixer_b = np.asarray(mixer_b, dtype=np.float32)

    # host-side folds of the weight-only algebra (O(d^2), negligible).
    # W1/W3 reshaped to the per-chunk layout w[p, k, n] = W[k*128+p, n].
    W1p = np.ascontiguousarray(
        mixer_w[0:D].reshape(KC, 128, D).transpose(1, 0, 2)).astype(np.float16)
    W2 = mixer_w[D:2 * D]
    W3p = np.ascontiguousarray(
        mixer_w[2 * D:3 * D].reshape(KC, 128, D).transpose(1, 0, 2)).astype(np.float16)
    W4 = mixer_w[3 * D:4 * D]
    w2v = (sim_w[:, 0] @ W2.astype(np.float32)).astype(ml_dtypes.bfloat16)[None, :]
    cor_vec = (cor_v_w[0] * cor_q_w[0, 0]).astype(np.float32)
    biasc = (cor_vec @ W4 + mixer_b).astype(np.float32)[None, :]
    qbias = ((1.0 - query_mask) * NEG_INF).astype(np.float32)
    vbias = np.log(video_mask + 1e-45).astype(np.float32)
    pw_row = np.ascontiguousarray(pool_w[:, 0])[None, :]  # [1, D]
    v16 = video_features.astype(np.float16)
    q16 = query_features.astype(np.float16)

    nc = _get_program()
    in_maps = []
    for c in range(NCORES):
        sl = slice(c * BPC, (c + 1) * BPC)
        in_maps.append({
            "v": np.ascontiguousarray(v16[sl]),
            "q": np.ascontiguousarray(q16[sl]),
            "qb": np.ascontiguousarray(qbias[sl]),
            "vb": np.ascontiguousarray(vbias[sl]),
            "w1": W1p,
            "w3": W3p,
            "w2v": w2v,
            "biasc": biasc,
            "pw": pw_row,
        })
    res = run_bass_kernel_spmd(nc, in_maps, core_ids=list(range(NCORES)))
    global _LAST_RESULTS
    _LAST_RESULTS = res
    out = np.concatenate([res.results[c]["out"] for c in range(NCORES)], axis=0)
    return out.astype(np.float32)


# revision 18
# speedup vs baseline: 1.3463x; 1.0344x over previous
"""Trainium2 Bass kernel for nn_FeatureRefinement.

Reference computation (bs=16, vl=1024, ql=64, d=1024):
    corr = einsum('bqd,bvd->bqv', Q, V); scores = softmax(corr, axis=1)
    corr_matrix = einsum('bqv,qd->bvd', scores, cor_w)     # cor_w constant over q
    sentence    = WeightedPool(Q)                           # (bs, d)
    sim         = cosine(V, sentence) + log(video_mask)     # (bs, vl)
    features    = concat([V, sim*sim_w, sentence_bcast, corr_matrix], -1)
    out         = relu(features @ mixer_w + mixer_b)

Algebraic restructuring (exact up to fp rounding):
  - softmax over q sums to 1  =>  corr_matrix[b,v,:] == cor_v_w*cor_q_w  (constant)
  - sim_features @ W2  == sim[b,v] * (sim_w.T @ W2)        (rank-1)
  - pooled_query @ W3  == sentence[b] @ W3                 (rank-1 per batch)
  so   out = relu(V @ W1 + [sim; 1; 1]^T @ [w2v; bias_hi; bias_lo])
  The only heavy compute is V @ W1 (4x FLOP reduction) plus O(bs*vl*d)
  vector work for the cosine similarity.

Sharding: data-parallel over batch, 2 batches per core on 8 cores. No
collectives; host scatters inputs / gathers outputs.

Implementation notes:
  - Query side runs in fp16; alpha = Q @ pool_w is one fused DVE op
    against a partition-broadcast pool_w row (no Q^T transposes).
  - Bias rows for both batches are computed in one M=2 matmul group and
    bounced through a DRAM scratch tile into the 3-partition augment rhs
    (engines cannot address partition offsets 1-2 directly).
  - A short stream of junk matmuls at t=0 warms the PE HAM clock gate
    (a cold PE runs at 1.2 GHz for its first ~3.4us of activity).
  - Output is stored fp16 and cast to fp32 on host (well within 2e-2).
  - DMA queue budget (per-queue, not per-link, is the constraint):
    sync carries V (4 MiB @ ~133 GB/s), gpsimd carries W1 (2 MiB @ ~173),
    scalar carries W3 + small tensors early and the fp16 stores late.
"""
import sys

sys.path.insert(0, "/opt/trn_rl_repo")

import numpy as np
import ml_dtypes
from contextlib import ExitStack

import concourse.bass as bass
import concourse.tile as tile
from concourse import bacc, mybir
from concourse.bass_utils import run_bass_kernel_spmd
from concourse.masks import make_identity


def _install_ntff_shim():
    """This container's antenv lacks axon_hooks; if tracing is requested
    (BASS_TRACE=1), run_bass_kernel_spmd would crash importing it. Provide
    the hook via trn_agent_boot's ctypes helper, and keep the trace
    post-processing local (no bucket uploads)."""
    import types
    try:
        import antenv  # noqa: F401
        import antenv.axon_hooks  # noqa: F401
        return  # already present
    except ImportError:
        pass
    try:
        import trn_agent_boot.trn_boot as _tb
        hook = _tb._ntff_profile_via_ctypes("/opt/axon/libaxon_pjrt.so")
        mod = types.ModuleType("antenv.axon_hooks")
        mod.get_axon_ntff_profile_hook = lambda: hook
        sys.modules["antenv.axon_hooks"] = mod
        from concourse import bass_utils as _bu
        _orig = _bu.upload_artifacts

        def _safe_upload(tmpdir):
            try:
                return _orig(tmpdir)
            except Exception:
                return f"file://{tmpdir}"

        _bu.upload_artifacts = _safe_upload
    except Exception:
        pass


_install_ntff_shim()

F32 = mybir.dt.float32
F16 = mybir.dt.float16
BF16 = mybir.dt.bfloat16
AF = mybir.ActivationFunctionType
AX = mybir.AxisListType
ALU = mybir.AluOpType

BS, VL, QL, D = 16, 1024, 64, 1024
NCORES = 8
BPC = BS // NCORES          # batches per core
KC = D // 128               # contraction chunks
SS = 512                    # v-rows per super-slab
NSS = VL // SS              # super-slabs per batch
NEG_INF = -1e30

VDT = F16                   # dtype of the heavy V @ W1 path


def _build_program():
    nc = bacc.Bacc("TRN2", target_bir_lowering=False, debug=False, num_devices=NCORES)

    v_d = nc.dram_tensor("v", [BPC, VL, D], VDT, kind="ExternalInput").ap()
    q_d = nc.dram_tensor("q", [BPC, QL, D], F16, kind="ExternalInput").ap()
    qb_d = nc.dram_tensor("qb", [BPC, QL], F32, kind="ExternalInput").ap()
    vb_d = nc.dram_tensor("vb", [BPC, VL], F32, kind="ExternalInput").ap()
    w1_d = nc.dram_tensor("w1", [KC, 128, D], VDT, kind="ExternalInput").ap()
    w3_d = nc.dram_tensor("w3", [KC, 128, D], VDT, kind="ExternalInput").ap()
    w2v_d = nc.dram_tensor("w2v", [1, D], BF16, kind="ExternalInput").ap()
    biasc_d = nc.dram_tensor("biasc", [1, D], F32, kind="ExternalInput").ap()
    pw_d = nc.dram_tensor("pw", [1, D], F32, kind="ExternalInput").ap()
    out_d = nc.dram_tensor("out", [BPC, VL, D], F16, kind="ExternalOutput").ap()

    with tile.TileContext(nc) as tc, ExitStack() as ctx:
        singles = ctx.enter_context(tc.tile_pool(name="singles", bufs=1))
        qstuff = ctx.enter_context(tc.tile_pool(name="qstuff", bufs=1))
        rows = ctx.enter_context(tc.tile_pool(name="rows", bufs=2))
        vload = ctx.enter_context(tc.tile_pool(name="vload", bufs=8))
        trashp = ctx.enter_context(tc.tile_pool(name="trashp", bufs=2))
        psA = ctx.enter_context(tc.tile_pool(name="psA", bufs=2, space="PSUM"))
        psOut = ctx.enter_context(tc.tile_pool(name="psOut", bufs=4, space="PSUM"))
        psRow = ctx.enter_context(tc.tile_pool(name="psRow", bufs=2, space="PSUM"))
        dramp = ctx.enter_context(tc.tile_pool(name="dramp", bufs=1, space="DRAM"))

        # ================= t=0 DMA issues ==========================
        # sync queue: all V row loads (consumed c1-order; vload recycles)
        v_sb_all = {}
        for b in range(BPC):
            for s4 in range(8):
                v_sb = vload.tile([128, D], VDT, tag="v_sb", name=f"v_{b}_{s4}")
                nc.sync.dma_start(out=v_sb, in_=v_d[b, s4 * 128:(s4 + 1) * 128, :])
                v_sb_all[(b, s4)] = v_sb

        # scalar queue: query-side smalls, then W3
        q_tiles = []
        for b in range(BPC):
            q_sb = qstuff.tile([QL, D], F16, name=f"q{b}")
            nc.scalar.dma_start(out=q_sb, in_=q_d[b])
            q_tiles.append(q_sb)
        pw_sb = singles.tile([1, D], F32)
        nc.scalar.dma_start(out=pw_sb, in_=pw_d)
        biasc_sb = singles.tile([1, D], F32)
        nc.scalar.dma_start(out=biasc_sb, in_=biasc_d)
        qb_sb = qstuff.tile([1, BPC, QL], F32)
        vb_sb = qstuff.tile([1, BPC, VL], F32)
        for b in range(BPC):
            nc.scalar.dma_start(out=qb_sb[:, b, :], in_=qb_d[b:b + 1, :])
            nc.scalar.dma_start(out=vb_sb[:, b, :], in_=vb_d[b:b + 1, :])
        aug3 = [qstuff.tile([3, D], BF16, name=f"aug3_{b}") for b in range(BPC)]
        for b in range(BPC):
            nc.scalar.dma_start(out=aug3[b][0:1, :], in_=w2v_d)
        w3_sb = singles.tile([128, KC, D], VDT)
        for k in range(KC):  # each chunk is one contiguous 256 KB read
            nc.scalar.dma_start(out=w3_sb[:, k, :], in_=w3_d[k])

        # gpsimd queue: W1
        w1_sb = singles.tile([128, KC, D], VDT)
        for k in range(KC):
            nc.gpsimd.dma_start(out=w1_sb[:, k, :], in_=w1_d[k])

        # ================= HAM warmup ==============================
        warm16 = singles.tile([128, 512], F16)
        nc.vector.memset(warm16, 0.0)
        for r in range(12):
            warm_ps = psOut.tile([128, 512], F32, tag="o_ps", name=f"warm{r}")
            nc.tensor.matmul(warm_ps, warm16[:, 0:128], warm16,
                             start=True, stop=True)

        # identities
        ident = singles.tile([128, 128], F32)
        make_identity(nc, ident)
        identH = singles.tile([128, 128], VDT)
        nc.vector.tensor_copy(identH, ident)

        # pool_w broadcast to QL partitions for the fused alpha op
        pw64 = singles.tile([QL, D], F32)
        nc.gpsimd.partition_broadcast(pw64, pw_sb)

        # ================= Phase A: query side =====================
        sentT2 = qstuff.tile([128, KC, BPC], VDT)    # sentence^T chunks
        snsq2 = qstuff.tile([1, BPC], F32)           # clamped ||sentence||^2

        for b in range(BPC):
            q_sb = q_tiles[b]
            # alpha[q] = sum_d Q[q,d]*pw[d]  (one fused DVE op)
            qtrash = trashp.tile([QL, D], F16, tag="qtrash")
            alpha_col = rows.tile([QL, 1], F32)
            nc.vector.scalar_tensor_tensor(
                out=qtrash, in0=q_sb, scalar=1.0, in1=pw64,
                op0=ALU.mult, op1=ALU.mult, accum_out=alpha_col)
            al_ps = psRow.tile([1, QL], F32, tag="row")
            nc.tensor.transpose(al_ps, alpha_col, ident[:QL, :QL])
            alpha_sb = rows.tile([1, QL], F32)
            nc.vector.tensor_add(alpha_sb, al_ps, qb_sb[:, b, :])

            # softmax over the free dim (1 partition)
            mx = rows.tile([1, 1], F32)
            nc.vector.reduce_max(mx, alpha_sb, axis=AX.X)
            asub = rows.tile([1, QL], F32)
            nc.vector.tensor_scalar_sub(asub, alpha_sb, mx)
            aexp = rows.tile([1, QL], F32)
            asum = rows.tile([1, 1], F32)
            nc.scalar.activation(aexp, asub, AF.Exp, accum_out=asum)
            rsum = rows.tile([1, 1], F32)
            nc.vector.reciprocal(rsum, asum)
            alphas_sb = rows.tile([1, QL], F32)
            nc.vector.tensor_scalar_mul(alphas_sb, aexp, rsum)

            # alphas^T : [QL, 1] fp16 (lhsT of the sentence matmul)
            alT_ps = psRow.tile([QL, 1], F32, tag="row")
            nc.tensor.transpose(alT_ps, alphas_sb, ident[:1, :1])
            alphasT_sb = rows.tile([QL, 1], F16)
            nc.vector.tensor_copy(alphasT_sb, alT_ps)

            # sentence = alphas @ Q : [1, D] fp32
            sent_sb = rows.tile([1, D], F32, tag="sent", bufs=1)
            for h in range(2):
                s_ps = psRow.tile([1, 512], F32, tag="row")
                nc.tensor.matmul(s_ps, alphasT_sb, q_sb[:, h * 512:(h + 1) * 512],
                                 start=True, stop=True)
                nc.vector.tensor_copy(sent_sb[:, h * 512:(h + 1) * 512], s_ps)

            # ||sentence||^2 clamped
            strash = rows.tile([1, D], F32, tag="strash", bufs=1)
            ssq = rows.tile([1, 1], F32)
            nc.scalar.activation(strash, sent_sb, AF.Square, accum_out=ssq)
            nc.vector.tensor_scalar_max(snsq2[:, b:b + 1], ssq, 1e-16)

            # sentence^T chunks: sentT2[p,k] = sent[k*128+p]
            sT_ps = psRow.tile([128, KC], F32, tag="row")
            for k in range(KC):
                nc.tensor.transpose(sT_ps[:, k:k + 1],
                                    sent_sb[:, k * 128:(k + 1) * 128],
                                    ident[:1, :1])
            nc.vector.tensor_copy(sentT2[:, :, b], sT_ps)

        # augment lhsT tiles: rows 1:3 are the constant ones
        aug_l = [[qstuff.tile([3, SS], BF16, name=f"augl_{b}_{s}")
                  for s in range(NSS)] for b in range(BPC)]
        for b in range(BPC):
            for s in range(NSS):
                nc.vector.memset(aug_l[b][s], 1.0)  # row 0 overwritten by sim

        def emit_bias_rows():
            # bias rows, both batches at once (M=2):
            #   bias_f[b] = sentence[b] @ W3 + biasc, split bf16 hi+lo
            biasc2 = singles.tile([BPC, D], F32)
            nc.gpsimd.partition_broadcast(biasc2, biasc_sb)
            bias_f = rows.tile([2, D], F32, tag="biasf", bufs=1)
            for h in range(2):
                b_ps = psRow.tile([2, 512], F32, tag="row")
                for k in range(KC):
                    nc.tensor.matmul(b_ps, sentT2[:, k, 0:BPC],
                                     w3_sb[:, k, h * 512:(h + 1) * 512],
                                     start=(k == 0), stop=(k == KC - 1))
                nc.vector.tensor_add(bias_f[:, h * 512:(h + 1) * 512], b_ps,
                                     biasc2[:, h * 512:(h + 1) * 512])
            bias_hi = rows.tile([2, D], BF16, tag="biashi", bufs=1)
            nc.vector.tensor_copy(bias_hi, bias_f)
            bias_lo = rows.tile([2, D], BF16, tag="biaslo", bufs=1)
            nc.vector.tensor_sub(bias_lo, bias_f, bias_hi)
            # engines can't write partitions 1:3 of aug3 directly; bounce the
            # bias rows through a DRAM scratch tile (DMA has no such limit)
            augd = dramp.tile([BPC, 2, D], BF16)
            nc.gpsimd.dma_start(out=augd[:, 0, :], in_=bias_hi)
            nc.gpsimd.dma_start(out=augd[:, 1, :], in_=bias_lo)
            for b in range(BPC):
                nc.gpsimd.dma_start(out=aug3[b][1:3, :], in_=augd[b])

        # ================= Phase C: video side (heavy) =============
        # Per-slab C1 (load+norm+transpose) immediately followed by that
        # slab's C2 (matmuls): the PE engine queue is in-order, so emitting
        # work whose inputs arrive late would head-of-line block it.
        vtpool = ctx.enter_context(tc.tile_pool(name="vtpool", bufs=4))
        opool = ctx.enter_context(tc.tile_pool(name="opool", bufs=3))

        for b in range(BPC):
            for s in range(NSS):
                # --- C1: row norms + transpose into vt
                vt = vtpool.tile([128, KC, SS], VDT, tag="vt", name=f"vt_{b}_{s}")
                vnsq_col = rows.tile([128, 4], F32, tag="vnsqc")
                for s4 in range(4):
                    v_sb = v_sb_all[(b, s * 4 + s4)]
                    vtrash = trashp.tile([128, D], F32, tag="vtrash")
                    nc.scalar.activation(vtrash, v_sb, AF.Square,
                                         accum_out=vnsq_col[:, s4:s4 + 1])
                    for g in range(2):
                        t_ps = psA.tile([128, 512], VDT, tag="tps")
                        for j in range(4):
                            k = g * 4 + j
                            nc.tensor.transpose(
                                t_ps[:, j * 128:(j + 1) * 128],
                                v_sb[:, k * 128:(k + 1) * 128], identH)
                        nc.vector.tensor_copy(
                            vt[:, g * 4:(g + 1) * 4, s4 * 128:(s4 + 1) * 128],
                            t_ps.rearrange("p (j c) -> p j c", j=4))

                if b == 0 and s == 0:
                    emit_bias_rows()

                # --- C2: sim row + main matmuls
                # dot row: sentence . V^T  -> [1, SS]
                dot_ps = psRow.tile([1, SS], F32, tag="row")
                for k in range(KC):
                    nc.tensor.matmul(dot_ps, sentT2[:, k, b:b + 1], vt[:, k, :],
                                     start=(k == 0), stop=(k == KC - 1))
                vnr_ps = psRow.tile([1, SS], F32, tag="row")
                for s4 in range(4):
                    nc.tensor.transpose(vnr_ps[:, s4 * 128:(s4 + 1) * 128],
                                        vnsq_col[:, s4:s4 + 1], ident)

                # sim = dot * rsqrt(max(vnsq,eps)*snsq) + log(video_mask)
                t1 = rows.tile([1, SS], F32, tag="t1")
                nc.vector.tensor_scalar(t1, vnr_ps, 1e-16, snsq2[:, b:b + 1],
                                        op0=ALU.max, op1=ALU.mult)
                t3 = rows.tile([1, SS], F32, tag="t3")
                nc.scalar.activation(t3, t1, AF.Abs_reciprocal_sqrt)
                t4 = rows.tile([1, SS], F32, tag="t4")
                nc.vector.tensor_mul(t4, dot_ps, t3)
                nc.vector.tensor_add(aug_l[b][s][0:1, :], t4,
                                     vb_sb[:, b, s * SS:(s + 1) * SS])

                for i in range(4):
                    out_sb = opool.tile([128, D], F16)
                    o_ps = [psOut.tile([128, 512], F32, tag="o_ps",
                                       name=f"o_ps_{b}_{s}_{i}_{h}")
                            for h in range(2)]
                    # keep 8 consecutive MMs on one PSUM bank: per-instruction
                    # bank alternation triggers the PE depth-cycling penalty
                    for h in range(2):
                        for k in range(KC):
                            nc.tensor.matmul(
                                o_ps[h], vt[:, k, i * 128:(i + 1) * 128],
                                w1_sb[:, k, h * 512:(h + 1) * 512],
                                start=(k == 0), stop=False)
                    for h in range(2):
                        nc.tensor.matmul(
                            o_ps[h], aug_l[b][s][:, i * 128:(i + 1) * 128],
                            aug3[b][:, h * 512:(h + 1) * 512],
                            start=False, stop=True)
                        # relu on DVE (fp16 store)
                        nc.vector.tensor_scalar_max(
                            out_sb[:, h * 512:(h + 1) * 512], o_ps[h], 0.0)
                    r0 = s * SS + i * 128
                    nc.scalar.dma_start(out=out_d[b, r0:r0 + 128, :], in_=out_sb)

    nc.compile()
    return nc


_NC = None
_LAST_RESULTS = None


def _get_program():
    global _NC
    if _NC is None:
        _NC = _build_program()
    return _NC


def kernel(video_features, query_features, video_mask, query_mask,
           sim_w, cor_v_w, cor_q_w, pool_w, mixer_w, mixer_b):
    video_features = np.asarray(video_features, dtype=np.float32)
    query_features = np.asarray(query_features, dtype=np.float32)
    video_mask = np.asarray(video_mask, dtype=np.float32)
    query_mask = np.asarray(query_mask, dtype=np.float32)
    sim_w = np.asarray(sim_w, dtype=np.float32)
    cor_v_w = np.asarray(cor_v_w, dtype=np.float32)
    cor_q_w = np.asarray(cor_q_w, dtype=np.float32)
    pool_w = np.asarray(pool_w, dtype=np.float32)
    mixer_w = np.asarray(mixer_w, dtype=np.float32)
    mixer_b = np.asarray(mixer_b, dtype=np.float32)

    # host-side folds of the weight-only algebra (O(d^2), negligible).
    # W1/W3 in chunk-major layout w[k, p, n] = W[k*128+p, n].
    W1p = np.ascontiguousarray(mixer_w[0:D].reshape(KC, 128, D)).astype(np.float16)
    W2 = mixer_w[D:2 * D]
    W3p = np.ascontiguousarray(mixer_w[2 * D:3 * D].reshape(KC, 128, D)).astype(np.float16)
    W4 = mixer_w[3 * D:4 * D]
    w2v = (sim_w[:, 0] @ W2.astype(np.float32)).astype(ml_dtypes.bfloat16)[None, :]
    cor_vec = (cor_v_w[0] * cor_q_w[0, 0]).astype(np.float32)
    biasc = (cor_vec @ W4 + mixer_b).astype(np.float32)[None, :]
    qbias = ((1.0 - query_mask) * NEG_INF).astype(np.float32)
    vbias = np.log(video_mask + 1e-45).astype(np.float32)
    pw_row = np.ascontiguousarray(pool_w[:, 0])[None, :]  # [1, D]
    v16 = video_features.astype(np.float16)
    q16 = query_features.astype(np.float16)

    nc = _get_program()
    in_maps = []
    for c in range(NCORES):
        sl = slice(c * BPC, (c + 1) * BPC)
        in_maps.append({
            "v": np.ascontiguousarray(v16[sl]),
            "q": np.ascontiguousarray(q16[sl]),
            "qb": np.ascontiguousarray(qbias[sl]),
            "vb": np.ascontiguousarray(vbias[sl]),
            "w1": W1p,
            "w3": W3p,
            "w2v": w2v,
            "biasc": biasc,
            "pw": pw_row,
        })
    res = run_bass_kernel_spmd(nc, in_maps, core_ids=list(range(NCORES)))
    global _LAST_RESULTS
    _LAST_RESULTS = res
    out = np.concatenate([res.results[c]["out"] for c in range(NCORES)], axis=0)
    return out.astype(np.float32)


# revision 21
# speedup vs baseline: 1.3750x; 1.0213x over previous
"""Trainium2 Bass kernel for nn_FeatureRefinement.

Reference computation (bs=16, vl=1024, ql=64, d=1024):
    corr = einsum('bqd,bvd->bqv', Q, V); scores = softmax(corr, axis=1)
    corr_matrix = einsum('bqv,qd->bvd', scores, cor_w)     # cor_w constant over q
    sentence    = WeightedPool(Q)                           # (bs, d)
    sim         = cosine(V, sentence) + log(video_mask)     # (bs, vl)
    features    = concat([V, sim*sim_w, sentence_bcast, corr_matrix], -1)
    out         = relu(features @ mixer_w + mixer_b)

Algebraic restructuring (exact up to fp rounding):
  - softmax over q sums to 1  =>  corr_matrix[b,v,:] == cor_v_w*cor_q_w  (constant)
  - sim_features @ W2  == sim[b,v] * (sim_w.T @ W2)        (rank-1)
  - pooled_query @ W3  == sentence[b] @ W3                 (rank-1 per batch)
  so   out = relu(V @ W1 + [sim; 1; 1]^T @ [w2v; bias_hi; bias_lo])
  The only heavy compute is V @ W1 (4x FLOP reduction) plus O(bs*vl*d)
  vector work for the cosine similarity.

Sharding: data-parallel over batch, 2 batches per core on 8 cores. No
collectives; host scatters inputs / gathers outputs.

Implementation notes:
  - Query side runs in fp16; alpha = Q @ pool_w is one fused DVE op
    against a partition-broadcast pool_w row (no Q^T transposes).
  - Bias rows for both batches are computed in one M=2 matmul group and
    bounced through a DRAM scratch tile into the 3-partition augment rhs
    (engines cannot address partition offsets 1-2 directly).
  - A short stream of junk matmuls at t=0 warms the PE HAM clock gate
    (a cold PE runs at 1.2 GHz for its first ~3.4us of activity).
  - Output is stored fp16 and cast to fp32 on host (well within 2e-2).
  - DMA queue budget (per-queue, not per-link, is the constraint):
    sync carries V (4 MiB @ ~133 GB/s), gpsimd carries W1 (2 MiB @ ~173),
    scalar carries W3 + small tensors early and the fp16 stores late.
"""
import sys

sys.path.insert(0, "/opt/trn_rl_repo")

import numpy as np
import ml_dtypes
from contextlib import ExitStack

import concourse.bass as bass
import concourse.tile as tile
from concourse import bacc, mybir
from concourse.bass_utils import run_bass_kernel_spmd
from concourse.masks import make_identity


def _install_ntff_shim():
    """This container's antenv lacks axon_hooks; if tracing is requested
    (BASS_TRACE=1), run_bass_kernel_spmd would crash importing it. Provide
    the hook via trn_agent_boot's ctypes helper, and keep the trace
    post-processing local (no bucket uploads)."""
    import types
    try:
        import antenv  # noqa: F401
        import antenv.axon_hooks  # noqa: F401
        return  # already present
    except ImportError:
        pass
    try:
        import trn_agent_boot.trn_boot as _tb
        hook = _tb._ntff_profile_via_ctypes("/opt/axon/libaxon_pjrt.so")
        mod = types.ModuleType("antenv.axon_hooks")
        mod.get_axon_ntff_profile_hook = lambda: hook
        sys.modules["antenv.axon_hooks"] = mod
        from concourse import bass_utils as _bu
        _orig = _bu.upload_artifacts

        def _safe_upload(tmpdir):
            try:
                return _orig(tmpdir)
            except Exception:
                return f"file://{tmpdir}"

        _bu.upload_artifacts = _safe_upload
    except Exception:
        pass


_install_ntff_shim()

F32 = mybir.dt.float32
F16 = mybir.dt.float16
BF16 = mybir.dt.bfloat16
AF = mybir.ActivationFunctionType
AX = mybir.AxisListType
ALU = mybir.AluOpType

BS, VL, QL, D = 16, 1024, 64, 1024
NCORES = 8
BPC = BS // NCORES          # batches per core
KC = D // 128               # contraction chunks
SS = 512                    # v-rows per super-slab
NSS = VL // SS              # super-slabs per batch
NEG_INF = -1e30

VDT = F16                   # dtype of the heavy V @ W1 path


def _build_program():
    nc = bacc.Bacc("TRN2", target_bir_lowering=False, debug=False, num_devices=NCORES)

    v_d = nc.dram_tensor("v", [BPC, VL, D], VDT, kind="ExternalInput").ap()
    q_d = nc.dram_tensor("q", [BPC, QL, D], F16, kind="ExternalInput").ap()
    qb_d = nc.dram_tensor("qb", [BPC, QL], F32, kind="ExternalInput").ap()
    vb_d = nc.dram_tensor("vb", [BPC, VL], F32, kind="ExternalInput").ap()
    w1_d = nc.dram_tensor("w1", [128, KC, D], VDT, kind="ExternalInput").ap()
    w3_d = nc.dram_tensor("w3", [128, KC, D], VDT, kind="ExternalInput").ap()
    w2v_d = nc.dram_tensor("w2v", [1, D], BF16, kind="ExternalInput").ap()
    biasc_d = nc.dram_tensor("biasc", [1, D], F32, kind="ExternalInput").ap()
    pw_d = nc.dram_tensor("pw", [1, D], F32, kind="ExternalInput").ap()
    out_d = nc.dram_tensor("out", [BPC, VL, D], F16, kind="ExternalOutput").ap()

    with tile.TileContext(nc) as tc, ExitStack() as ctx:
        singles = ctx.enter_context(tc.tile_pool(name="singles", bufs=1))
        qstuff = ctx.enter_context(tc.tile_pool(name="qstuff", bufs=1))
        rows = ctx.enter_context(tc.tile_pool(name="rows", bufs=2))
        vload = ctx.enter_context(tc.tile_pool(name="vload", bufs=8))
        trashp = ctx.enter_context(tc.tile_pool(name="trashp", bufs=2))
        psA = ctx.enter_context(tc.tile_pool(name="psA", bufs=2, space="PSUM"))
        psOut = ctx.enter_context(tc.tile_pool(name="psOut", bufs=4, space="PSUM"))
        psRow = ctx.enter_context(tc.tile_pool(name="psRow", bufs=2, space="PSUM"))
        dramp = ctx.enter_context(tc.tile_pool(name="dramp", bufs=1, space="DRAM"))

        # ================= t=0 DMA issues ==========================
        # W1/W3 are host-laid-out partition-major ([128, KC*D] per row), so
        # one DMA moves each with 16 KiB per-partition lines (queues are
        # packet-rate-limited; packet = one per-partition contiguous run).
        # sync queue: batch-0 V rows; gpsimd: W1 then batch-1 V rows;
        # scalar: query smalls + W3 early, fp16 out stores later.
        v_sb_all = {}
        for s4 in range(8):
            v_sb = vload.tile([128, D], VDT, tag="v_sb", name=f"v_0_{s4}")
            nc.sync.dma_start(out=v_sb, in_=v_d[0, s4 * 128:(s4 + 1) * 128, :])
            v_sb_all[(0, s4)] = v_sb

        w1_sb = singles.tile([128, KC, D], VDT)
        nc.gpsimd.dma_start(out=w1_sb, in_=w1_d)
        for s4 in range(8):
            v_sb = vload.tile([128, D], VDT, tag="v_sb", name=f"v_1_{s4}")
            nc.gpsimd.dma_start(out=v_sb, in_=v_d[1, s4 * 128:(s4 + 1) * 128, :])
            v_sb_all[(1, s4)] = v_sb

        q_tiles = []
        for b in range(BPC):
            q_sb = qstuff.tile([QL, D], F16, name=f"q{b}")
            nc.scalar.dma_start(out=q_sb, in_=q_d[b])
            q_tiles.append(q_sb)
        pw_sb = singles.tile([1, D], F32)
        nc.scalar.dma_start(out=pw_sb, in_=pw_d)
        qb_sb = qstuff.tile([1, BPC, QL], F32)
        for b in range(BPC):
            nc.scalar.dma_start(out=qb_sb[:, b, :], in_=qb_d[b:b + 1, :])
        w3_sb = singles.tile([128, KC, D], VDT)
        nc.scalar.dma_start(out=w3_sb, in_=w3_d)
        biasc_sb = singles.tile([1, D], F32)
        nc.scalar.dma_start(out=biasc_sb, in_=biasc_d)
        vb_sb = qstuff.tile([1, BPC, VL], F32)
        for b in range(BPC):
            nc.scalar.dma_start(out=vb_sb[:, b, :], in_=vb_d[b:b + 1, :])
        aug3 = [qstuff.tile([3, D], BF16, name=f"aug3_{b}") for b in range(BPC)]
        for b in range(BPC):
            nc.scalar.dma_start(out=aug3[b][0:1, :], in_=w2v_d)

        # ================= HAM warmup ==============================
        warm16 = singles.tile([128, 512], F16)
        nc.vector.memset(warm16, 0.0)
        for r in range(12):
            warm_ps = psOut.tile([128, 512], F32, tag="o_ps", name=f"warm{r}")
            nc.tensor.matmul(warm_ps, warm16[:, 0:128], warm16,
                             start=True, stop=True)

        # identities
        ident = singles.tile([128, 128], F32)
        make_identity(nc, ident)
        identH = singles.tile([128, 128], VDT)
        nc.vector.tensor_copy(identH, ident)

        # pool_w broadcast to QL partitions for the fused alpha op
        pw64 = singles.tile([QL, D], F32)
        nc.gpsimd.partition_broadcast(pw64, pw_sb)

        # ================= Phase A: query side =====================
        sentT2 = qstuff.tile([128, KC, BPC], VDT)    # sentence^T chunks
        snsq2 = qstuff.tile([1, BPC], F32)           # clamped ||sentence||^2

        for b in range(BPC):
            q_sb = q_tiles[b]
            # alpha[q] = sum_d Q[q,d]*pw[d]  (one fused DVE op)
            qtrash = trashp.tile([QL, D], F16, tag="qtrash")
            alpha_col = rows.tile([QL, 1], F32)
            nc.vector.scalar_tensor_tensor(
                out=qtrash, in0=q_sb, scalar=1.0, in1=pw64,
                op0=ALU.mult, op1=ALU.mult, accum_out=alpha_col)
            al_ps = psRow.tile([1, QL], F32, tag="row")
            nc.tensor.transpose(al_ps, alpha_col, ident[:QL, :QL])
            alpha_sb = rows.tile([1, QL], F32)
            nc.vector.tensor_add(alpha_sb, al_ps, qb_sb[:, b, :])

            # softmax over the free dim (1 partition)
            mx = rows.tile([1, 1], F32)
            nc.vector.reduce_max(mx, alpha_sb, axis=AX.X)
            asub = rows.tile([1, QL], F32)
            nc.vector.tensor_scalar_sub(asub, alpha_sb, mx)
            aexp = rows.tile([1, QL], F32)
            asum = rows.tile([1, 1], F32)
            nc.scalar.activation(aexp, asub, AF.Exp, accum_out=asum)
            rsum = rows.tile([1, 1], F32)
            nc.vector.reciprocal(rsum, asum)
            alphas_sb = rows.tile([1, QL], F32)
            nc.vector.tensor_scalar_mul(alphas_sb, aexp, rsum)

            # alphas^T : [QL, 1] fp16 (lhsT of the sentence matmul)
            alT_ps = psRow.tile([QL, 1], F32, tag="row")
            nc.tensor.transpose(alT_ps, alphas_sb, ident[:1, :1])
            alphasT_sb = rows.tile([QL, 1], F16)
            nc.vector.tensor_copy(alphasT_sb, alT_ps)

            # sentence = alphas @ Q : [1, D] fp32
            sent_sb = rows.tile([1, D], F32, tag="sent", bufs=1)
            for h in range(2):
                s_ps = psRow.tile([1, 512], F32, tag="row")
                nc.tensor.matmul(s_ps, alphasT_sb, q_sb[:, h * 512:(h + 1) * 512],
                                 start=True, stop=True)
                nc.vector.tensor_copy(sent_sb[:, h * 512:(h + 1) * 512], s_ps)

            # ||sentence||^2 clamped
            strash = rows.tile([1, D], F32, tag="strash", bufs=1)
            ssq = rows.tile([1, 1], F32)
            nc.scalar.activation(strash, sent_sb, AF.Square, accum_out=ssq)
            nc.vector.tensor_scalar_max(snsq2[:, b:b + 1], ssq, 1e-16)

            # sentence^T chunks: sentT2[p,k] = sent[k*128+p]
            sT_ps = psRow.tile([128, KC], F32, tag="row")
            for k in range(KC):
                nc.tensor.transpose(sT_ps[:, k:k + 1],
                                    sent_sb[:, k * 128:(k + 1) * 128],
                                    ident[:1, :1])
            nc.vector.tensor_copy(sentT2[:, :, b], sT_ps)

        # augment lhsT tiles: rows 1:3 are the constant ones
        aug_l = [[qstuff.tile([3, SS], BF16, name=f"augl_{b}_{s}")
                  for s in range(NSS)] for b in range(BPC)]
        for b in range(BPC):
            for s in range(NSS):
                nc.vector.memset(aug_l[b][s], 1.0)  # row 0 overwritten by sim

        def emit_bias_rows():
            # bias rows, both batches at once (M=2):
            #   bias_f[b] = sentence[b] @ W3 + biasc, split bf16 hi+lo
            biasc2 = singles.tile([BPC, D], F32)
            nc.gpsimd.partition_broadcast(biasc2, biasc_sb)
            bias_f = rows.tile([2, D], F32, tag="biasf", bufs=1)
            for h in range(2):
                b_ps = psRow.tile([2, 512], F32, tag="row")
                for k in range(KC):
                    nc.tensor.matmul(b_ps, sentT2[:, k, 0:BPC],
                                     w3_sb[:, k, h * 512:(h + 1) * 512],
                                     start=(k == 0), stop=(k == KC - 1))
                nc.vector.tensor_add(bias_f[:, h * 512:(h + 1) * 512], b_ps,
                                     biasc2[:, h * 512:(h + 1) * 512])
            bias_hi = rows.tile([2, D], BF16, tag="biashi", bufs=1)
            nc.vector.tensor_copy(bias_hi, bias_f)
            bias_lo = rows.tile([2, D], BF16, tag="biaslo", bufs=1)
            nc.vector.tensor_sub(bias_lo, bias_f, bias_hi)
            # engines can't write partitions 1:3 of aug3 directly; bounce the
            # bias rows through a DRAM scratch tile (DMA has no such limit)
            augd = dramp.tile([BPC, 2, D], BF16)
            nc.gpsimd.dma_start(out=augd[:, 0, :], in_=bias_hi)
            nc.gpsimd.dma_start(out=augd[:, 1, :], in_=bias_lo)
            for b in range(BPC):
                nc.gpsimd.dma_start(out=aug3[b][1:3, :], in_=augd[b])

        # ================= Phase C: video side (heavy) =============
        # Per-slab C1 (load+norm+transpose) immediately followed by that
        # slab's C2 (matmuls): the PE engine queue is in-order, so emitting
        # work whose inputs arrive late would head-of-line block it.
        vtpool = ctx.enter_context(tc.tile_pool(name="vtpool", bufs=4))
        opool = ctx.enter_context(tc.tile_pool(name="opool", bufs=3))

        for b in range(BPC):
            for s in range(NSS):
                # --- C1: row norms + transpose into vt
                vt = vtpool.tile([128, KC, SS], VDT, tag="vt", name=f"vt_{b}_{s}")
                vnsq_col = rows.tile([128, 4], F32, tag="vnsqc")
                for s4 in range(4):
                    v_sb = v_sb_all[(b, s * 4 + s4)]
                    vtrash = trashp.tile([128, D], F32, tag="vtrash")
                    nc.scalar.activation(vtrash, v_sb, AF.Square,
                                         accum_out=vnsq_col[:, s4:s4 + 1])
                    for g in range(2):
                        t_ps = psA.tile([128, 512], VDT, tag="tps")
                        for j in range(4):
                            k = g * 4 + j
                            nc.tensor.transpose(
                                t_ps[:, j * 128:(j + 1) * 128],
                                v_sb[:, k * 128:(k + 1) * 128], identH)
                        nc.vector.tensor_copy(
                            vt[:, g * 4:(g + 1) * 4, s4 * 128:(s4 + 1) * 128],
                            t_ps.rearrange("p (j c) -> p j c", j=4))

                if b == 0 and s == 0:
                    emit_bias_rows()

                # --- C2: sim row + main matmuls
                # dot row: sentence . V^T  -> [1, SS]
                dot_ps = psRow.tile([1, SS], F32, tag="row")
                for k in range(KC):
                    nc.tensor.matmul(dot_ps, sentT2[:, k, b:b + 1], vt[:, k, :],
                                     start=(k == 0), stop=(k == KC - 1))
                vnr_ps = psRow.tile([1, SS], F32, tag="row")
                for s4 in range(4):
                    nc.tensor.transpose(vnr_ps[:, s4 * 128:(s4 + 1) * 128],
                                        vnsq_col[:, s4:s4 + 1], ident)

                # sim = dot * rsqrt(max(vnsq,eps)*snsq) + log(video_mask)
                t1 = rows.tile([1, SS], F32, tag="t1")
                nc.vector.tensor_scalar(t1, vnr_ps, 1e-16, snsq2[:, b:b + 1],
                                        op0=ALU.max, op1=ALU.mult)
                t3 = rows.tile([1, SS], F32, tag="t3")
                nc.scalar.activation(t3, t1, AF.Abs_reciprocal_sqrt)
                t4 = rows.tile([1, SS], F32, tag="t4")
                nc.vector.tensor_mul(t4, dot_ps, t3)
                nc.vector.tensor_add(aug_l[b][s][0:1, :], t4,
                                     vb_sb[:, b, s * SS:(s + 1) * SS])

                for i in range(4):
                    out_sb = opool.tile([128, D], F16)
                    o_ps = [psOut.tile([128, 512], F32, tag="o_ps",
                                       name=f"o_ps_{b}_{s}_{i}_{h}")
                            for h in range(2)]
                    # keep 8 consecutive MMs on one PSUM bank: per-instruction
                    # bank alternation triggers the PE depth-cycling penalty
                    for h in range(2):
                        for k in range(KC):
                            nc.tensor.matmul(
                                o_ps[h], vt[:, k, i * 128:(i + 1) * 128],
                                w1_sb[:, k, h * 512:(h + 1) * 512],
                                start=(k == 0), stop=False)
                    for h in range(2):
                        nc.tensor.matmul(
                            o_ps[h], aug_l[b][s][:, i * 128:(i + 1) * 128],
                            aug3[b][:, h * 512:(h + 1) * 512],
                            start=False, stop=True)
                        # relu on DVE (fp16 store)
                        nc.vector.tensor_scalar_max(
                            out_sb[:, h * 512:(h + 1) * 512], o_ps[h], 0.0)
                    r0 = s * SS + i * 128
                    nc.scalar.dma_start(out=out_d[b, r0:r0 + 128, :], in_=out_sb)

    nc.compile()
    return nc


_NC = None
_LAST_RESULTS = None


def _get_program():
    global _NC
    if _NC is None:
        _NC = _build_program()
    return _NC


def kernel(video_features, query_features, video_mask, query_mask,
           sim_w, cor_v_w, cor_q_w, pool_w, mixer_w, mixer_b):
    video_features = np.asarray(video_features, dtype=np.float32)
    query_features = np.asarray(query_features, dtype=np.float32)
    video_mask = np.asarray(video_mask, dtype=np.float32)
    query_mask = np.asarray(query_mask, dtype=np.float32)
    sim_w = np.asarray(sim_w, dtype=np.float32)
    cor_v_w = np.asarray(cor_v_w, dtype=np.float32)
    cor_q_w = np.asarray(cor_q_w, dtype=np.float32)
    pool_w = np.asarray(pool_w, dtype=np.float32)
    mixer_w = np.asarray(mixer_w, dtype=np.float32)
    mixer_b = np.asarray(mixer_b, dtype=np.float32)

    # host-side folds of the weight-only algebra (O(d^2), negligible).
    # W1/W3 in partition-major layout w[p, k, n] = W[k*128+p, n] so the
    # whole tensor loads as one DMA with 16 KiB per-partition lines.
    W1p = np.ascontiguousarray(
        mixer_w[0:D].reshape(KC, 128, D).transpose(1, 0, 2)).astype(np.float16)
    W2 = mixer_w[D:2 * D]
    W3p = np.ascontiguousarray(
        mixer_w[2 * D:3 * D].reshape(KC, 128, D).transpose(1, 0, 2)).astype(np.float16)
    W4 = mixer_w[3 * D:4 * D]
    w2v = (sim_w[:, 0] @ W2.astype(np.float32)).astype(ml_dtypes.bfloat16)[None, :]
    cor_vec = (cor_v_w[0] * cor_q_w[0, 0]).astype(np.float32)
    biasc = (cor_vec @ W4 + mixer_b).astype(np.float32)[None, :]
    qbias = ((1.0 - query_mask) * NEG_INF).astype(np.float32)
    vbias = np.log(video_mask + 1e-45).astype(np.float32)
    pw_row = np.ascontiguousarray(pool_w[:, 0])[None, :]  # [1, D]
    v16 = video_features.astype(np.float16)
    q16 = query_features.astype(np.float16)

    nc = _get_program()
    in_maps = []
    for c in range(NCORES):
        sl = slice(c * BPC, (c + 1) * BPC)
        in_maps.append({
            "v": np.ascontiguousarray(v16[sl]),
            "q": np.ascontiguousarray(q16[sl]),
            "qb": np.ascontiguousarray(qbias[sl]),
            "vb": np.ascontiguousarray(vbias[sl]),
            "w1": W1p,
            "w3": W3p,
            "w2v": w2v,
            "biasc": biasc,
            "pw": pw_row,
        })
    res = run_bass_kernel_spmd(nc, in_maps, core_ids=list(range(NCORES)))
    global _LAST_RESULTS
    _LAST_RESULTS = res
    out = np.concatenate([res.results[c]["out"] for c in range(NCORES)], axis=0)
    return out.astype(np.float32)


# revision 30
# speedup vs baseline: 1.4018x; 1.0195x over previous
"""Trainium2 Bass kernel for nn_FeatureRefinement.

Reference computation (bs=16, vl=1024, ql=64, d=1024):
    corr = einsum('bqd,bvd->bqv', Q, V); scores = softmax(corr, axis=1)
    corr_matrix = einsum('bqv,qd->bvd', scores, cor_w)     # cor_w constant over q
    sentence    = WeightedPool(Q)                           # (bs, d)
    sim         = cosine(V, sentence) + log(video_mask)     # (bs, vl)
    features    = concat([V, sim*sim_w, sentence_bcast, corr_matrix], -1)
    out         = relu(features @ mixer_w + mixer_b)

Algebraic restructuring (exact up to fp rounding):
  - softmax over q sums to 1  =>  corr_matrix[b,v,:] == cor_v_w*cor_q_w  (constant)
  - sim_features @ W2  == sim[b,v] * (sim_w.T @ W2)        (rank-1)
  - pooled_query @ W3  == sentence[b] @ W3                 (rank-1 per batch)
  so   out = relu(V @ W1 + [sim; 1; 1]^T @ [w2v; bias_hi; bias_lo])
  The only heavy compute is V @ W1 (4x FLOP reduction) plus O(bs*vl*d)
  vector work for the cosine similarity.

Sharding: data-parallel over batch, 2 batches per core on 8 cores. No
collectives; host scatters inputs / gathers outputs.

Implementation notes:
  - Query side runs in fp16; alpha = Q @ pool_w is one fused DVE op
    against a partition-broadcast pool_w row (no Q^T transposes).
  - Bias rows for both batches are computed in one M=2 matmul group and
    bounced through a DRAM scratch tile into the 3-partition augment rhs
    (engines cannot address partition offsets 1-2 directly).
  - A short stream of junk matmuls at t=0 warms the PE HAM clock gate
    (a cold PE runs at 1.2 GHz for its first ~3.4us of activity).
  - Output is stored fp16 and cast to fp32 on host (well within 2e-2).
  - DMA queue budget (per-queue, not per-link, is the constraint):
    sync carries V (4 MiB @ ~133 GB/s), gpsimd carries W1 (2 MiB @ ~173),
    scalar carries W3 + small tensors early and the fp16 stores late.
"""
import sys

sys.path.insert(0, "/opt/trn_rl_repo")

import numpy as np
import ml_dtypes
from contextlib import ExitStack

import concourse.bass as bass
import concourse.tile as tile
from concourse import bacc, mybir
from concourse.bass_utils import run_bass_kernel_spmd
from concourse.masks import make_identity


def _install_ntff_shim():
    """This container's antenv lacks axon_hooks; if tracing is requested
    (BASS_TRACE=1), run_bass_kernel_spmd would crash importing it. Provide
    the hook via trn_agent_boot's ctypes helper, and keep the trace
    post-processing local (no bucket uploads)."""
    import types
    try:
        import antenv  # noqa: F401
        import antenv.axon_hooks  # noqa: F401
        return  # already present
    except ImportError:
        pass
    try:
        import trn_agent_boot.trn_boot as _tb
        hook = _tb._ntff_profile_via_ctypes("/opt/axon/libaxon_pjrt.so")
        mod = types.ModuleType("antenv.axon_hooks")
        mod.get_axon_ntff_profile_hook = lambda: hook
        sys.modules["antenv.axon_hooks"] = mod
        from concourse import bass_utils as _bu
        _orig = _bu.upload_artifacts

        def _safe_upload(tmpdir):
            try:
                return _orig(tmpdir)
            except Exception:
                return f"file://{tmpdir}"

        _bu.upload_artifacts = _safe_upload
    except Exception:
        pass


_install_ntff_shim()

F32 = mybir.dt.float32
F16 = mybir.dt.float16
BF16 = mybir.dt.bfloat16
AF = mybir.ActivationFunctionType
AX = mybir.AxisListType
ALU = mybir.AluOpType

BS, VL, QL, D = 16, 1024, 64, 1024
NCORES = 8
BPC = BS // NCORES          # batches per core
KC = D // 128               # contraction chunks
SS = 512                    # v-rows per super-slab
NSS = VL // SS              # super-slabs per batch
NEG_INF = -1e30

VDT = F16                   # dtype of the heavy V @ W1 path


def _build_program():
    nc = bacc.Bacc("TRN2", target_bir_lowering=False, debug=False, num_devices=NCORES)

    v_d = nc.dram_tensor("v", [BPC, VL, D], VDT, kind="ExternalInput").ap()
    q_d = nc.dram_tensor("q", [BPC, QL, D], F16, kind="ExternalInput").ap()
    w1_d = nc.dram_tensor("w1", [128, KC, D], VDT, kind="ExternalInput").ap()
    w3_d = nc.dram_tensor("w3", [128, KC, D], VDT, kind="ExternalInput").ap()
    # pool_w pre-broadcast to QL partitions (device partition_broadcast
    # costs a ~14us gpsimd custom-op library load)
    pw_d = nc.dram_tensor("pw", [QL, D], F16, kind="ExternalInput").ap()
    biasc2_d = nc.dram_tensor("biasc2", [BPC, D], F32, kind="ExternalInput").ap()
    # packed smalls (one DMA): [qb(2*64) | vb(2*1024) | w2v(1024)]
    packA_d = nc.dram_tensor("packA", [1, 3200], F32, kind="ExternalInput").ap()
    out_d = nc.dram_tensor("out", [BPC, VL, D], F16, kind="ExternalOutput").ap()

    with tile.TileContext(nc) as tc, ExitStack() as ctx:
        singles = ctx.enter_context(tc.tile_pool(name="singles", bufs=1))
        qstuff = ctx.enter_context(tc.tile_pool(name="qstuff", bufs=1))
        rows = ctx.enter_context(tc.tile_pool(name="rows", bufs=2))
        vload = ctx.enter_context(tc.tile_pool(name="vload", bufs=8))
        trashp = ctx.enter_context(tc.tile_pool(name="trashp", bufs=2))
        psA = ctx.enter_context(tc.tile_pool(name="psA", bufs=2, space="PSUM"))
        psOut = ctx.enter_context(tc.tile_pool(name="psOut", bufs=4, space="PSUM"))
        psRow = ctx.enter_context(tc.tile_pool(name="psRow", bufs=2, space="PSUM"))
        dramp = ctx.enter_context(tc.tile_pool(name="dramp", bufs=1, space="DRAM"))

        # ================= t=0 DMA issues ==========================
        # DMA completion semaphores are a shared pool of ~8: more than that
        # many in-flight DMAs serialize in waves. Consolidate transfers.
        # W1/W3 are host-laid-out partition-major so one DMA moves each
        # with 16 KiB per-partition lines.
        # sync: batch-0 V; gpsimd: W1 then batch-1 V; scalar: q/pw/packs,
        # W3, then the fp16 out stores later.
        v_pieces = {}   # (b, half) -> [128, 2, D] tile holding 2 row-chunks
        for half in range(4):
            v_sb = vload.tile([128, 2, D], VDT, tag="v_sb", name=f"v_0_{half}")
            nc.sync.dma_start(
                out=v_sb,
                in_=v_d[0, half * 256:(half + 1) * 256, :].rearrange(
                    "(j p) d -> p j d", j=2))
            v_pieces[(0, half)] = v_sb

        w1_sb = singles.tile([128, KC, D], VDT)
        nc.gpsimd.dma_start(out=w1_sb, in_=w1_d)
        for half in range(4):
            v_sb = vload.tile([128, 2, D], VDT, tag="v_sb", name=f"v_1_{half}")
            nc.gpsimd.dma_start(
                out=v_sb,
                in_=v_d[1, half * 256:(half + 1) * 256, :].rearrange(
                    "(j p) d -> p j d", j=2))
            v_pieces[(1, half)] = v_sb

        def v_chunk(b, s4):  # [128, D] view of chunk s4 (0..7) of batch b
            return v_pieces[(b, s4 // 2)][:, s4 % 2, :]

        q_sb2 = qstuff.tile([QL, BPC, D], F16)
        nc.scalar.dma_start(out=q_sb2, in_=q_d.rearrange("b q d -> q b d"))
        pw64 = singles.tile([QL, D], F16)
        nc.scalar.dma_start(out=pw64, in_=pw_d)
        packA = singles.tile([1, 3200], F32)
        nc.scalar.dma_start(out=packA, in_=packA_d)
        w3_sb = singles.tile([128, KC, D], VDT)
        nc.scalar.dma_start(out=w3_sb, in_=w3_d)
        biasc2 = singles.tile([BPC, D], F32)
        nc.scalar.dma_start(out=biasc2, in_=biasc2_d)

        def qb_row(b):
            return packA[:, b * QL:(b + 1) * QL]

        def vb_row(b, lo, hi):
            return packA[:, 2 * QL + b * VL + lo:2 * QL + b * VL + hi]

        w2v_row = packA[:, 2 * QL + 2 * VL:2 * QL + 2 * VL + D]

        aug3 = [qstuff.tile([3, D], BF16, name=f"aug3_{b}") for b in range(BPC)]
        for b in range(BPC):
            nc.vector.tensor_copy(aug3[b][0:1, :], w2v_row)

        # ================= HAM warmup ==============================
        warm16 = singles.tile([128, 512], F16)
        nc.vector.memset(warm16, 0.0)
        for r in range(12):
            warm_ps = psOut.tile([128, 512], F32, tag="o_ps", name=f"warm{r}")
            nc.tensor.matmul(warm_ps, warm16[:, 0:128], warm16,
                             start=True, stop=True)

        # identities
        ident = singles.tile([128, 128], F32)
        make_identity(nc, ident)
        identH = singles.tile([128, 128], VDT)
        nc.vector.tensor_copy(identH, ident)

        # ================= Phase A: query side =====================
        sentT2 = qstuff.tile([128, KC, BPC], VDT)    # sentence^T chunks
        snsq2 = qstuff.tile([1, BPC], F32)           # clamped ||sentence||^2

        for b in range(BPC):
            q_sb = q_sb2[:, b, :]
            # alpha[q] = sum_d Q[q,d]*pw[d]  (one fused DVE op)
            qtrash = trashp.tile([QL, D], F16, tag="qtrash")
            alpha_col = rows.tile([QL, 1], F32)
            nc.vector.scalar_tensor_tensor(
                out=qtrash, in0=q_sb, scalar=1.0, in1=pw64,
                op0=ALU.mult, op1=ALU.mult, accum_out=alpha_col)
            al_ps = psRow.tile([1, QL], F32, tag="row")
            nc.tensor.transpose(al_ps, alpha_col, ident[:QL, :QL])
            alpha_sb = rows.tile([1, QL], F32)
            nc.vector.tensor_add(alpha_sb, al_ps, qb_row(b))

            # softmax over the free dim (1 partition)
            mx = rows.tile([1, 1], F32)
            nc.vector.reduce_max(mx, alpha_sb, axis=AX.X)
            asub = rows.tile([1, QL], F32)
            nc.vector.tensor_scalar_sub(asub, alpha_sb, mx)
            aexp = rows.tile([1, QL], F32)
            asum = rows.tile([1, 1], F32)
            nc.scalar.activation(aexp, asub, AF.Exp, accum_out=asum)
            rsum = rows.tile([1, 1], F32)
            nc.vector.reciprocal(rsum, asum)
            alphas_sb = rows.tile([1, QL], F32)
            nc.vector.tensor_scalar_mul(alphas_sb, aexp, rsum)

            # alphas^T : [QL, 1] fp16 (lhsT of the sentence matmul)
            alT_ps = psRow.tile([QL, 1], F32, tag="row")
            nc.tensor.transpose(alT_ps, alphas_sb, ident[:1, :1])
            alphasT_sb = rows.tile([QL, 1], F16)
            nc.vector.tensor_copy(alphasT_sb, alT_ps)

            # sentence = alphas @ Q : [1, D] fp32
            sent_sb = rows.tile([1, D], F32, tag="sent", bufs=1)
            for h in range(2):
                s_ps = psRow.tile([1, 512], F32, tag="row")
                nc.tensor.matmul(s_ps, alphasT_sb, q_sb[:, h * 512:(h + 1) * 512],
                                 start=True, stop=True)
                nc.vector.tensor_copy(sent_sb[:, h * 512:(h + 1) * 512], s_ps)

            # ||sentence||^2 clamped
            strash = rows.tile([1, D], F32, tag="strash", bufs=1)
            ssq = rows.tile([1, 1], F32)
            nc.scalar.activation(strash, sent_sb, AF.Square, accum_out=ssq)
            nc.vector.tensor_scalar_max(snsq2[:, b:b + 1], ssq, 1e-16)

            # sentence^T chunks: sentT2[p,k] = sent[k*128+p]
            sT_ps = psRow.tile([128, KC], F32, tag="row")
            for k in range(KC):
                nc.tensor.transpose(sT_ps[:, k:k + 1],
                                    sent_sb[:, k * 128:(k + 1) * 128],
                                    ident[:1, :1])
            nc.vector.tensor_copy(sentT2[:, :, b], sT_ps)

        # augment lhsT tiles: rows 1:3 are the constant ones
        aug_l = [[qstuff.tile([3, SS], BF16, name=f"augl_{b}_{s}")
                  for s in range(NSS)] for b in range(BPC)]
        for b in range(BPC):
            for s in range(NSS):
                nc.vector.memset(aug_l[b][s], 1.0)  # row 0 overwritten by sim

        def emit_bias_rows():
            # bias rows, both batches at once (M=2):
            #   bias_f[b] = sentence[b] @ W3 + biasc, split bf16 hi+lo
            bias_f = rows.tile([2, D], F32, tag="biasf", bufs=1)
            for h in range(2):
                b_ps = psRow.tile([2, 512], F32, tag="row")
                for k in range(KC):
                    nc.tensor.matmul(b_ps, sentT2[:, k, 0:BPC],
                                     w3_sb[:, k, h * 512:(h + 1) * 512],
                                     start=(k == 0), stop=(k == KC - 1))
                nc.vector.tensor_add(bias_f[:, h * 512:(h + 1) * 512], b_ps,
                                     biasc2[:, h * 512:(h + 1) * 512])
            bias_hi = rows.tile([2, D], BF16, tag="biashi", bufs=1)
            nc.vector.tensor_copy(bias_hi, bias_f)
            bias_lo = rows.tile([2, D], BF16, tag="biaslo", bufs=1)
            nc.vector.tensor_sub(bias_lo, bias_f, bias_hi)
            # engines can't write partitions 1:3 of aug3 directly; bounce the
            # bias rows through a DRAM scratch tile (DMA has no such limit)
            augd = dramp.tile([BPC, 2, D], BF16)
            nc.gpsimd.dma_start(out=augd[:, 0, :], in_=bias_hi)
            nc.gpsimd.dma_start(out=augd[:, 1, :], in_=bias_lo)
            for b in range(BPC):
                nc.gpsimd.dma_start(out=aug3[b][1:3, :], in_=augd[b])

        # ================= Phase C: video side (heavy) =============
        # Per-slab C1 (load+norm+transpose) immediately followed by that
        # slab's C2 (matmuls): the PE engine queue is in-order, so emitting
        # work whose inputs arrive late would head-of-line block it.
        vtpool = ctx.enter_context(tc.tile_pool(name="vtpool", bufs=4))
        opool = ctx.enter_context(tc.tile_pool(name="opool", bufs=2))

        for b in range(BPC):
            for s in range(NSS):
                # --- C1: row norms + transpose into vt
                vt = vtpool.tile([128, KC, SS], VDT, tag="vt", name=f"vt_{b}_{s}")
                vnsq_col = rows.tile([128, 4], F32, tag="vnsqc")
                for s4 in range(4):
                    v_sb = v_chunk(b, s * 4 + s4)
                    vtrash = trashp.tile([128, D], F32, tag="vtrash")
                    nc.scalar.activation(vtrash, v_sb, AF.Square,
                                         accum_out=vnsq_col[:, s4:s4 + 1])
                    for g in range(2):
                        t_ps = psA.tile([128, 512], VDT, tag="tps")
                        for j in range(4):
                            k = g * 4 + j
                            nc.tensor.transpose(
                                t_ps[:, j * 128:(j + 1) * 128],
                                v_sb[:, k * 128:(k + 1) * 128], identH)
                        nc.vector.tensor_copy(
                            vt[:, g * 4:(g + 1) * 4, s4 * 128:(s4 + 1) * 128],
                            t_ps.rearrange("p (j c) -> p j c", j=4))

                if b == 0 and s == 0:
                    emit_bias_rows()

                # --- C2: sim row + main matmuls
                # dot row: sentence . V^T  -> [1, SS]
                dot_ps = psRow.tile([1, SS], F32, tag="row")
                for k in range(KC):
                    nc.tensor.matmul(dot_ps, sentT2[:, k, b:b + 1], vt[:, k, :],
                                     start=(k == 0), stop=(k == KC - 1))
                vnr_ps = psRow.tile([1, SS], F32, tag="row")
                for s4 in range(4):
                    nc.tensor.transpose(vnr_ps[:, s4 * 128:(s4 + 1) * 128],
                                        vnsq_col[:, s4:s4 + 1], ident)

                # sim = dot * rsqrt(max(vnsq,eps)*snsq) + log(video_mask)
                t1 = rows.tile([1, SS], F32, tag="t1")
                nc.vector.tensor_scalar(t1, vnr_ps, 1e-16, snsq2[:, b:b + 1],
                                        op0=ALU.max, op1=ALU.mult)
                t3 = rows.tile([1, SS], F32, tag="t3")
                nc.scalar.activation(t3, t1, AF.Abs_reciprocal_sqrt)
                t4 = rows.tile([1, SS], F32, tag="t4")
                nc.vector.tensor_mul(t4, dot_ps, t3)
                nc.vector.tensor_add(aug_l[b][s][0:1, :], t4,
                                     vb_row(b, s * SS, (s + 1) * SS))

                out_sb = opool.tile([128, 4, D], F16)  # whole slab, 1 store
                for i in range(4):
                    o_ps = [psOut.tile([128, 512], F32, tag="o_ps",
                                       name=f"o_ps_{b}_{s}_{i}_{h}")
                            for h in range(2)]
                    # keep 8 consecutive MMs on one PSUM bank: per-instruction
                    # bank alternation triggers the PE depth-cycling penalty
                    for h in range(2):
                        for k in range(KC):
                            nc.tensor.matmul(
                                o_ps[h], vt[:, k, i * 128:(i + 1) * 128],
                                w1_sb[:, k, h * 512:(h + 1) * 512],
                                start=(k == 0), stop=False)
                    for h in range(2):
                        nc.tensor.matmul(
                            o_ps[h], aug_l[b][s][:, i * 128:(i + 1) * 128],
                            aug3[b][:, h * 512:(h + 1) * 512],
                            start=False, stop=True)
                        # relu on DVE (fp16 store)
                        nc.vector.tensor_scalar_max(
                            out_sb[:, i, h * 512:(h + 1) * 512], o_ps[h], 0.0)
                nc.scalar.dma_start(
                    out=out_d[b, s * SS:(s + 1) * SS, :].rearrange(
                        "(i p) d -> p i d", i=4),
                    in_=out_sb)

    nc.compile()
    return nc


_NC = None
_LAST_RESULTS = None


def _get_program():
    global _NC
    if _NC is None:
        _NC = _build_program()
    return _NC


def kernel(video_features, query_features, video_mask, query_mask,
           sim_w, cor_v_w, cor_q_w, pool_w, mixer_w, mixer_b):
    video_features = np.asarray(video_features, dtype=np.float32)
    query_features = np.asarray(query_features, dtype=np.float32)
    video_mask = np.asarray(video_mask, dtype=np.float32)
    query_mask = np.asarray(query_mask, dtype=np.float32)
    sim_w = np.asarray(sim_w, dtype=np.float32)
    cor_v_w = np.asarray(cor_v_w, dtype=np.float32)
    cor_q_w = np.asarray(cor_q_w, dtype=np.float32)
    pool_w = np.asarray(pool_w, dtype=np.float32)
    mixer_w = np.asarray(mixer_w, dtype=np.float32)
    mixer_b = np.asarray(mixer_b, dtype=np.float32)

    # host-side folds of the weight-only algebra (O(d^2), negligible).
    # W1/W3 in partition-major layout w[p, k, n] = W[k*128+p, n] so the
    # whole tensor loads as one DMA with 16 KiB per-partition lines.
    W1p = np.ascontiguousarray(
        mixer_w[0:D].reshape(KC, 128, D).transpose(1, 0, 2)).astype(np.float16)
    W2 = mixer_w[D:2 * D]
    W3p = np.ascontiguousarray(
        mixer_w[2 * D:3 * D].reshape(KC, 128, D).transpose(1, 0, 2)).astype(np.float16)
    W4 = mixer_w[3 * D:4 * D]
    w2v = (sim_w[:, 0] @ W2.astype(np.float32)).astype(np.float32)
    cor_vec = (cor_v_w[0] * cor_q_w[0, 0]).astype(np.float32)
    biasc = (cor_vec @ W4 + mixer_b).astype(np.float32)
    biasc2 = np.ascontiguousarray(np.broadcast_to(biasc, (BPC, D)))
    qbias = ((1.0 - query_mask) * NEG_INF).astype(np.float32)
    vbias = np.log(video_mask + 1e-45).astype(np.float32)
    pw64 = np.ascontiguousarray(
        np.broadcast_to(pool_w[:, 0], (QL, D))).astype(np.float16)
    v16 = video_features.astype(np.float16)
    q16 = query_features.astype(np.float16)

    nc = _get_program()
    in_maps = []
    for c in range(NCORES):
        sl = slice(c * BPC, (c + 1) * BPC)
        packA = np.concatenate(
            [qbias[sl].reshape(-1), vbias[sl].reshape(-1), w2v])[None, :]
        in_maps.append({
            "v": np.ascontiguousarray(v16[sl]),
            "q": np.ascontiguousarray(q16[sl]),
            "w1": W1p,
            "w3": W3p,
            "pw": pw64,
            "biasc2": biasc2,
            "packA": np.ascontiguousarray(packA),
        })
    res = run_bass_kernel_spmd(nc, in_maps, core_ids=list(range(NCORES)))
    global _LAST_RESULTS
    _LAST_RESULTS = res
    out = np.concatenate([res.results[c]["out"] for c in range(NCORES)], axis=0)
    return out.astype(np.float32)
